# revision 18
# baseline (speedup 1.0000x reference)
"""Trainium2 Bass kernel for nn_DiscreteNarrator (GRU + VQ codebook + heads).

Strategy: data-parallel over batch across 8 NeuronCores (64 rows/core).
Per core:
  - GRU recurrence runs fully on-chip. Layout: gh[b, g] with batch on
    PSUM partitions, gate slices on the free dim; dual column-group
    matmuls (tile_position) pack two independent 512-wide gate slices
    into one 128x128 PE pass so the 64-row batch uses the full array.
  - Matmul inputs in fp16 (10-bit mantissa; verified 0 argmin flips vs
    fp32 reference on the generated codebook margins), fp32 PSUM
    accumulation. The x-projection for step t+1 is fused into step t's
    PE stream (no separate gi_all phase, no DRAM staging).
  - VQ tail in fp32: queries, scores S = 2 q.e - |e|^2 - 1e-9*n (argmax
    == argmin-with-first-index-tie-break), one-hot matmul against the
    fp32 codebook so quantized rows are exact; heads use fp16 weights.
vq_loss partial sums are reduced on the host (the only cross-core term).
"""
import os
import sys
import types

import numpy as np

_HERE = os.path.dirname(os.path.abspath(__file__))


def _register_ntff_hook():
    """Make trace=True (BASS_TRACE=1) work under axon if the hook is absent."""
    try:
        from antenv.axon_hooks import get_axon_ntff_profile_hook  # noqa
        return
    except Exception:
        pass
    try:
        from trn_agent_boot.trn_boot import _ntff_profile_via_ctypes
        hook = _ntff_profile_via_ctypes("/opt/axon/libaxon_pjrt.so")
        mod = types.ModuleType("antenv.axon_hooks")
        mod.get_axon_ntff_profile_hook = lambda: hook
        mod.set_axon_ntff_profile_hook = lambda h: None
        import antenv
        sys.modules["antenv.axon_hooks"] = mod
        antenv.axon_hooks = mod
    except Exception:
        pass


_register_ntff_hook()

import concourse.tile as tile
import concourse.mybir as mybir
from concourse import bacc
from concourse.bass_utils import run_bass_kernel_spmd

F32 = mybir.dt.float32
F16 = mybir.dt.float16
U32 = mybir.dt.uint32
I32 = mybir.dt.int32
AF = mybir.ActivationFunctionType
OP = mybir.AluOpType

# problem dims (hardcoded per contract)
B, W, D, H = 512, 64, 512, 1024
NCODE, K, C = 1024, 8, 64
BN = K * C          # 512
G3 = 3 * H          # 3072
NCORES = 8
BS = B // NCORES    # 64

_CACHE = {}


def _hmap(c):
    """Free-dim offset of hT chunk c inside the [128, 512] hT tile."""
    return (c % 4) * 128 + (c // 4) * 64


def _build():
    from contextlib import ExitStack

    nc = bacc.Bacc(None, target_bir_lowering=False)

    # ---- inputs ----
    xT_hi = nc.dram_tensor("xT_hi", [128, 4, W * BS], F16, kind="ExternalInput")
    xT_lo = nc.dram_tensor("xT_lo", [128, 4, W * BS], F16, kind="ExternalInput")
    wihT_hi = nc.dram_tensor("wihT_hi", [128, 4, G3], F16, kind="ExternalInput")
    wihT_lo = nc.dram_tensor("wihT_lo", [128, 4, G3], F16, kind="ExternalInput")
    whhT_hi = nc.dram_tensor("whhT_hi", [128, 8, G3], F16, kind="ExternalInput")
    whhT_lo = nc.dram_tensor("whhT_lo", [128, 8, G3], F16, kind="ExternalInput")
    bt_r = nc.dram_tensor("bt_r", [128, 512], F32, kind="ExternalInput")
    bt_z = nc.dram_tensor("bt_z", [128, 512], F32, kind="ExternalInput")
    bt_nh = nc.dram_tensor("bt_nh", [128, 512], F32, kind="ExternalInput")
    bt_ni = nc.dram_tensor("bt_ni", [128, 512], F32, kind="ExternalInput")
    wqT = nc.dram_tensor("wqT", [128, 8, BN], F32, kind="ExternalInput")
    bq_row = nc.dram_tensor("bq_row", [1, BN], F32, kind="ExternalInput")
    embT2 = nc.dram_tensor("embT2", [128, NCODE], F32, kind="ExternalInput")
    negnorm = nc.dram_tensor("negnorm", [1, NCODE], F32, kind="ExternalInput")
    emb8 = nc.dram_tensor("emb8", [128, 8, C], F32, kind="ExternalInput")
    lwT16 = nc.dram_tensor("lwT16", [128, 4, BS], F16, kind="ExternalInput")
    uw1T = nc.dram_tensor("uw1T", [128, 8, H], F16, kind="ExternalInput")
    ub1_row = nc.dram_tensor("ub1_row", [1, H], F16, kind="ExternalInput")
    uw2T = nc.dram_tensor("uw2T", [128, 8, H], F16, kind="ExternalInput")
    ub2_row = nc.dram_tensor("ub2_row", [1, H], F16, kind="ExternalInput")
    pw1T = nc.dram_tensor("pw1T", [128, 4, H], F16, kind="ExternalInput")
    pb1_row = nc.dram_tensor("pb1_row", [1, H], F16, kind="ExternalInput")
    pw2T = nc.dram_tensor("pw2T", [128, 8, BN], F16, kind="ExternalInput")
    pb2_row = nc.dram_tensor("pb2_row", [1, BN], F16, kind="ExternalInput")
    ones16 = nc.dram_tensor("ones16", [1, 128], F16, kind="ExternalInput")
    ones32 = nc.dram_tensor("ones32", [1, 128], F32, kind="ExternalInput")
    onescol = nc.dram_tensor("onescol", [128, 1], F32, kind="ExternalInput")
    eye16 = nc.dram_tensor("eye16", [128, 128], F16, kind="ExternalInput")
    eye32 = nc.dram_tensor("eye32", [128, 128], F32, kind="ExternalInput")
    iota8 = nc.dram_tensor("iota8", [128, 8], F32, kind="ExternalInput")

    # ---- outputs ----
    o_idx = nc.dram_tensor("o_idx", [BS, K], I32, kind="ExternalOutput")
    o_qst = nc.dram_tensor("o_qst", [BS, K, C], F32, kind="ExternalOutput")
    o_narr = nc.dram_tensor("o_narr", [BS, BN], F32, kind="ExternalOutput")
    o_unc = nc.dram_tensor("o_unc", [BS, H], F32, kind="ExternalOutput")
    o_pred = nc.dram_tensor("o_pred", [BS, BN], F32, kind="ExternalOutput")
    o_loss = nc.dram_tensor("o_loss", [1, 1], F32, kind="ExternalOutput")
    o_hid = nc.dram_tensor("o_hid", [BS, H], F32, kind="ExternalOutput")

    with tile.TileContext(nc) as tc, ExitStack() as stack:
        cpool = stack.enter_context(tc.tile_pool(name="cpool", bufs=1))
        dyn2 = stack.enter_context(tc.tile_pool(name="dyn2", bufs=2))
        dyng = stack.enter_context(tc.tile_pool(name="dyng", bufs=8))
        dynx = stack.enter_context(tc.tile_pool(name="dynx", bufs=3))

        # consts
        on16_sb = cpool.tile([1, 128], F16, tag="on16")
        on32_sb = cpool.tile([1, 128], F32, tag="on32")
        onc_sb = cpool.tile([128, 1], F32, tag="onc")
        eye16_sb = cpool.tile([128, 128], F16, tag="eye16")
        eye32_sb = cpool.tile([128, 128], F32, tag="eye32")
        iota_sb = cpool.tile([128, 8], F32, tag="iota")
        btr_sb = cpool.tile([128, 512], F32, tag="btr")
        btz_sb = cpool.tile([128, 512], F32, tag="btz")
        btnh_sb = cpool.tile([128, 512], F32, tag="btnh")
        btni_sb = cpool.tile([128, 512], F32, tag="btni")
        zrow16 = cpool.tile([1, 512], F16, tag="zrow16")
        nc.vector.memset(zrow16[:], 0.0)
        zrow32 = cpool.tile([1, 512], F32, tag="zrow32")
        nc.vector.memset(zrow32[:], 0.0)
        for dst, src in ((on16_sb, ones16), (on32_sb, ones32), (onc_sb, onescol),
                         (eye16_sb, eye16), (eye32_sb, eye32), (iota_sb, iota8),
                         (btr_sb, bt_r), (btz_sb, bt_z), (btnh_sb, bt_nh),
                         (btni_sb, bt_ni)):
            nc.sync.dma_start(dst[:], src[:])

        # ---------------- recurrence ----------------
        h2_prev = dyn2.tile([128, 512], F32, tag="h2")
        nc.vector.memset(h2_prev[:], 0.0)

        xt_tiles = {}

        def prefetch_xt(t):
            if t > W:
                return
            xh = dynx.tile([128, 4, BS], F16, tag="xth")
            nc.sync.dma_start(xh[:], xT_hi[:, :, (t - 1) * BS:t * BS])
            xl = dynx.tile([128, 4, BS], F16, tag="xtl")
            nc.sync.dma_start(xl[:], xT_lo[:, :, (t - 1) * BS:t * BS])
            xt_tiles[t] = (xh, xl)

        with tc.tile_pool(name="rw", bufs=1) as rw:
            wih_hi = rw.tile([128, 4, G3], F16, tag="wihhi")
            wih_lo = rw.tile([128, 4, G3], F16, tag="wihlo")
            whh_hi = rw.tile([128, 8, G3], F16, tag="whhhi")
            whh_lo = rw.tile([128, 8, G3], F16, tag="whhlo")
            nc.sync.dma_start(wih_hi[:], wihT_hi[:])
            nc.sync.dma_start(wih_lo[:], wihT_lo[:])
            nc.sync.dma_start(whh_hi[:], whhT_hi[:])
            nc.sync.dma_start(whh_lo[:], whhT_lo[:])

            with tc.tile_pool(name="ghp", bufs=2, space="PSUM") as ghp, \
                 tc.tile_pool(name="smp", bufs=2, space="PSUM") as smp:

                def a_block(t, first_step):
                    """x-projection matmuls (3-term hi/lo) for step t."""
                    xh, xl = xt_tiles.pop(t)
                    gh = ghp.tile([128, 1536], F32, tag="gh")
                    i_n = smp.tile([128, 512], F32, tag="sm")
                    nc.vector.memset(gh[:], 0.0)
                    nc.vector.memset(i_n[:], 0.0)
                    terms = ((xh, wih_hi), (xh, wih_lo), (xl, wih_hi))
                    for col in range(2):
                        po = col * 64
                        cs = col * 512
                        for ti, (xa, wa) in enumerate(terms):
                            for c in range(4):
                                last = (ti == 2 and c == 3)
                                nc.tensor.matmul(
                                    gh[po:po + 64, 0:512], xa[:, c, :],
                                    wa[:, c, cs:cs + 512],
                                    start=False, stop=(last and first_step),
                                    skip_group_check=True,
                                    tile_position=(0, po))
                                nc.tensor.matmul(
                                    gh[po:po + 64, 512:1024], xa[:, c, :],
                                    wa[:, c, 1024 + cs:1024 + cs + 512],
                                    start=False, stop=(last and first_step),
                                    skip_group_check=True,
                                    tile_position=(0, po))
                                nc.tensor.matmul(
                                    i_n[po:po + 64, :], xa[:, c, :],
                                    wa[:, c, 2048 + cs:2048 + cs + 512],
                                    start=False, stop=last,
                                    skip_group_check=True,
                                    tile_position=(0, po))
                    return gh, i_n

                def b_block(gh, hT_prev):
                    """h-recurrence matmuls (3-term hi/lo). Region order: r, hn, z."""
                    def hchunk(c, lo):
                        off = 512 * lo + _hmap(c)
                        return hT_prev[:, off:off + 64]
                    for g0, fr in ((0, 0), (2048, 1024), (1024, 512)):
                        for ti in range(3):
                            wa = whh_lo if ti == 1 else whh_hi
                            hlo = 1 if ti == 2 else 0
                            for c in range(8):
                                hc = hchunk(c, hlo)
                                for col in range(2):
                                    po = col * 64
                                    nc.tensor.matmul(
                                        gh[po:po + 64, fr:fr + 512], hc,
                                        wa[:, c, g0 + col * 512:g0 + col * 512 + 512],
                                        start=False,
                                        stop=(ti == 2 and c == 7),
                                        skip_group_check=True,
                                        tile_position=(0, po))

                def c_block(gh, i_n, h2_in, t):
                    last = (t == W)
                    r_pre = dyng.tile([128, 512], F32, tag="gate")
                    nc.vector.tensor_tensor(r_pre[:], gh[:, 0:512], btr_sb[:], OP.add)
                    r_sb = dyng.tile([128, 512], F32, tag="gate")
                    nc.scalar.activation(r_sb[:], r_pre[:], AF.Sigmoid)
                    t2a = dyng.tile([128, 512], F32, tag="gate")
                    nc.vector.tensor_tensor(t2a[:], i_n[:], btni_sb[:], OP.add)
                    t1 = dyng.tile([128, 512], F32, tag="gate")
                    if t == 1:
                        nc.vector.tensor_tensor(t1[:], btnh_sb[:], r_sb[:], OP.mult)
                    else:
                        t0 = dyng.tile([128, 512], F32, tag="gate")
                        nc.vector.tensor_tensor(t0[:], gh[:, 1024:1536], btnh_sb[:], OP.add)
                        nc.vector.tensor_tensor(t1[:], t0[:], r_sb[:], OP.mult)
                    t2 = dyng.tile([128, 512], F32, tag="gate")
                    nc.vector.tensor_tensor(t2[:], t1[:], t2a[:], OP.add)
                    n_sb = dyng.tile([128, 512], F32, tag="gate")
                    nc.scalar.activation(n_sb[:], t2[:], AF.Tanh)
                    z_pre = dyng.tile([128, 512], F32, tag="gate")
                    nc.vector.tensor_tensor(z_pre[:], gh[:, 512:1024], btz_sb[:], OP.add)
                    z_sb = dyng.tile([128, 512], F32, tag="gate")
                    nc.scalar.activation(z_sb[:], z_pre[:], AF.Sigmoid)
                    v_sb = dyng.tile([128, 512], F32, tag="gate")
                    nc.scalar.activation(v_sb[:], z_pre[:], AF.Sigmoid, scale=-1.0)
                    u_sb = dyng.tile([128, 512], F32, tag="gate")
                    nc.vector.tensor_tensor(u_sb[:], z_sb[:], h2_in[:], OP.mult)
                    w_sb = dyng.tile([128, 512], F32, tag="gate")
                    nc.vector.tensor_tensor(w_sb[:], v_sb[:], n_sb[:], OP.mult)
                    h2_t = dyn2.tile([128, 512], F32, tag="h2")
                    nc.vector.tensor_tensor(h2_t[:], u_sb[:], w_sb[:], OP.add)
                    if last:
                        return h2_t, None, None
                    h2b_hi = dyn2.tile([128, 512], F16, tag="h2bh")
                    nc.scalar.copy(h2b_hi[:], h2_t[:])
                    h2b_lo = dyn2.tile([128, 512], F16, tag="h2bl")
                    nc.vector.tensor_tensor(h2b_lo[:], h2_t[:], h2b_hi[:], OP.subtract)
                    return h2_t, h2b_hi, h2b_lo

                def d_block(h2b_hi, h2b_lo):
                    tr = smp.tile([128, 1024], F16, tag="sm")
                    for j in range(4):
                        nc.tensor.transpose(tr[:, j * 128:(j + 1) * 128],
                                            h2b_hi[:, j * 128:(j + 1) * 128], eye16_sb[:])
                    for j in range(4):
                        nc.tensor.transpose(tr[:, 512 + j * 128:512 + (j + 1) * 128],
                                            h2b_lo[:, j * 128:(j + 1) * 128], eye16_sb[:])
                    hT_t = dyn2.tile([128, 1024], F16, tag="hT")
                    nc.scalar.copy(hT_t[:], tr[:])
                    return hT_t

                prefetch_xt(1)
                prefetch_xt(2)
                gh_t, in_t = a_block(1, True)
                hT_prev = None
                for t in range(1, W + 1):
                    prefetch_xt(t + 2)
                    if t > 1:
                        b_block(gh_t, hT_prev)
                    h2_t, h2b_hi, h2b_lo = c_block(gh_t, in_t, h2_prev, t)
                    if t < W:
                        gh_t, in_t = a_block(t + 1, False)
                        hT_prev = d_block(h2b_hi, h2b_lo)
                    h2_prev = h2_t

        h2f = h2_prev  # [128, 512] f32: p<64 -> h[b, 0:512]; p>=64 -> h[b, 512:1024]
        nc.sync.dma_start(o_hid[:, 0:512], h2f[0:64, :])
        nc.sync.dma_start(o_hid[:, 512:1024], h2f[64:128, :])

        # tail weights (allocated after the recurrence weights free their space)
        tailp = stack.enter_context(tc.tile_pool(name="tailp", bufs=1))
        wq_sb = tailp.tile([128, 8, BN], F32, tag="wq")
        uw1_sb = tailp.tile([128, 8, H], F16, tag="uw1")
        embT2_sb = tailp.tile([128, NCODE], F32, tag="embT2")
        nn_sb = tailp.tile([1, NCODE], F32, tag="negnorm")
        emb8_sb = tailp.tile([128, 8, C], F32, tag="emb8")
        lw_sb = tailp.tile([128, 4, BS], F16, tag="lw")
        uw2_sb = tailp.tile([128, 8, H], F16, tag="uw2")
        pw1_sb = tailp.tile([128, 4, H], F16, tag="pw1")
        pw2_sb = tailp.tile([128, 8, BN], F16, tag="pw2")
        bq_sb = tailp.tile([1, BN], F32, tag="bq")
        ub1_sb = tailp.tile([1, H], F16, tag="ub1")
        ub2_sb = tailp.tile([1, H], F16, tag="ub2")
        pb1_sb = tailp.tile([1, H], F16, tag="pb1")
        pb2_sb = tailp.tile([1, BN], F16, tag="pb2")
        for dst, src in ((wq_sb, wqT), (uw1_sb, uw1T), (embT2_sb, embT2),
                         (nn_sb, negnorm), (emb8_sb, emb8), (lw_sb, lwT16),
                         (uw2_sb, uw2T), (pw1_sb, pw1T), (pw2_sb, pw2T),
                         (bq_sb, bq_row), (ub1_sb, ub1_row), (ub2_sb, ub2_row),
                         (pb1_sb, pb1_row), (pb2_sb, pb2_row)):
            nc.sync.dma_start(dst[:], src[:])

        # ---------------- tail ----------------
        with tc.tile_pool(name="pstail", bufs=1, space="PSUM") as pst, \
             tc.tile_pool(name="pstr1", bufs=1, space="PSUM") as pstr1, \
             tc.tile_pool(name="pstr2", bufs=2, space="PSUM") as pstr2, \
             tc.tile_pool(name="dscr", bufs=1, space="DRAM") as dscr:

            # fp32 transposed final hidden
            trh = pstr1.tile([128, 512], F32, tag="tr")
            for j in range(4):
                nc.tensor.transpose(trh[:, j * 128:(j + 1) * 128],
                                    h2f[:, j * 128:(j + 1) * 128], eye32_sb[:])
            hT32 = tailp.tile([128, 512], F32, tag="hT32")
            nc.vector.tensor_copy(hT32[:], trh[:])

            # queries q = h @ Wq.T + bq  [64, 512]
            q_ps = pst.tile([128, BN], F32, tag="pq")
            nc.tensor.matmul(q_ps[0:64, :], on32_sb[0:1, 0:64], bq_sb[0:1, :],
                             start=True, stop=False)
            for c in range(8):
                nc.tensor.matmul(q_ps[0:64, :], hT32[:, _hmap(c):_hmap(c) + 64],
                                 wq_sb[:, c, :], start=False, stop=(c == 7))
            q_sb = tailp.tile([128, BN], F32, tag="q")
            nc.vector.tensor_copy(q_sb[0:64, :], q_ps[0:64, :])

            # qT chunks: [128, 4, 64], chunk j partitions = (k=2j: c | k=2j+1: c)
            trq = pstr1.tile([128, 4, C], F32, tag="trq")
            for j in range(4):
                nc.tensor.transpose(trq[:, j, :], q_sb[0:64, j * 128:(j + 1) * 128],
                                    eye32_sb[0:64, 0:64])
            qT_sb = tailp.tile([128, 4, C], F32, tag="qT")
            nc.vector.tensor_copy(qT_sb[:], trq[:])

            # scores S_j = 2 q.e - |e|^2 - 1e-9 n  [128 (2 k), 1024 codes]
            idx_bounce = dscr.tile([1, 512], F32, tag="idxb")
            for j in range(4):
                S_ps = pst.tile([128, NCODE], F32, tag="pt")
                for ns in range(2):
                    nc.tensor.matmul(
                        S_ps[:, ns * 512:(ns + 1) * 512],
                        on32_sb[0:1, 0:128],
                        nn_sb[0:1, ns * 512:(ns + 1) * 512],
                        start=True, stop=False, skip_group_check=True)
                    for col in range(2):
                        po = col * 64
                        nc.tensor.matmul(
                            S_ps[po:po + 64, ns * 512:(ns + 1) * 512],
                            qT_sb[po:po + 64, j, :],
                            embT2_sb[po:po + 64, ns * 512:(ns + 1) * 512],
                            start=False, stop=True, skip_group_check=True,
                            tile_position=(po, po))
                S_sb = tailp.tile([128, NCODE], F32, tag="S")
                if j % 2 == 0:
                    nc.vector.tensor_copy(S_sb[:], S_ps[:])
                else:
                    nc.scalar.copy(S_sb[:], S_ps[:])
                mx = tailp.tile([128, 8], F32, tag="mx")
                mi = tailp.tile([128, 8], U32, tag="mi")
                nc.vector.max_with_indices(mx[:], mi[:], S_sb[:])
                nc.sync.dma_start(o_idx[:, 2 * j:2 * j + 1], mi[0:64, 0:1].bitcast(I32))
                nc.sync.dma_start(o_idx[:, 2 * j + 1:2 * j + 2], mi[64:128, 0:1].bitcast(I32))
                idxf = tailp.tile([128, 1], F32, tag="idxf")
                nc.vector.tensor_copy(idxf[:], mi[:, 0:1])
                nc.sync.dma_start(idx_bounce[0:1, j * 128:(j + 1) * 128], idxf[:, 0:1])

            idxrow = tailp.tile([1, 512], F32, tag="idxrow")
            nc.sync.dma_start(idxrow[:], idx_bounce[:])

            # broadcast idx over partitions; q index order along free = (k, b)
            bc_ps = pst.tile([128, 512], F32, tag="pq")
            nc.tensor.matmul(bc_ps[:], on32_sb[0:1, :], idxrow[0:1, :],
                             start=True, stop=True)
            bc_sb = dyng.tile([128, 512], F32, tag="gate")
            nc.vector.tensor_copy(bc_sb[:], bc_ps[:])

            # one-hot matmul -> narr [64, (k c)] (exact fp32 emb rows)
            narr_ps = pst.tile([128, BN], F32, tag="pq")
            nc.tensor.matmul(narr_ps[0:64, :], on32_sb[0:1, 0:64], zrow32[0:1, :],
                             start=True, stop=False)
            for nci in range(8):
                oh = dyng.tile([128, 512], F32, tag="gate")
                nc.vector.tensor_scalar(oh[:], bc_sb[:], iota_sb[:, nci:nci + 1], None,
                                        op0=OP.is_equal)
                for k in range(8):
                    nc.tensor.matmul(narr_ps[0:64, k * C:(k + 1) * C],
                                     oh[:, k * 64:(k + 1) * 64], emb8_sb[:, nci, :],
                                     start=False, stop=(nci == 7 and k == 7))
            narr_sb = tailp.tile([128, BN], F32, tag="narr")
            nc.vector.tensor_copy(narr_sb[0:64, :], narr_ps[0:64, :])
            nc.sync.dma_start(o_qst.rearrange("b k c -> b (k c)"), narr_sb[0:64, :])
            nc.sync.dma_start(o_narr[:], narr_sb[0:64, :])

            # vq loss partial: sum((q - narr)^2)
            d_sb = dyng.tile([128, BN], F32, tag="gate")
            nc.vector.tensor_tensor(d_sb[0:64, :], q_sb[0:64, :], narr_sb[0:64, :],
                                    OP.subtract)
            d2 = dyng.tile([128, BN], F32, tag="gate")
            dcol = tailp.tile([128, 1], F32, tag="dcol")
            nc.vector.scalar_tensor_tensor(d2[0:64, :], d_sb[0:64, :], 0.0,
                                           d_sb[0:64, :], op0=OP.add, op1=OP.mult,
                                           accum_out=dcol[0:64, :])
            loss_ps = pst.tile([128, 512], F32, tag="pq")
            nc.tensor.matmul(loss_ps[0:1, 0:1], dcol[0:64, 0:1], onc_sb[0:64, 0:1],
                             start=True, stop=True)
            loss_sb = tailp.tile([1, 1], F32, tag="loss")
            nc.vector.tensor_copy(loss_sb[:], loss_ps[0:1, 0:1])
            nc.sync.dma_start(o_loss[:], loss_sb[:])

            # narrT fp16 chunks for the heads
            narr16 = tailp.tile([128, BN], F16, tag="narr16")
            nc.scalar.copy(narr16[0:64, :], narr_sb[0:64, :])
            trn = pstr2.tile([128, 4, C], F16, tag="tr16")
            for j in range(4):
                nc.tensor.transpose(trn[:, j, :], narr16[0:64, j * 128:(j + 1) * 128],
                                    eye16_sb[0:64, 0:64])
            narrT = tailp.tile([128, 4, C], F16, tag="narrT")
            nc.vector.tensor_copy(narrT[:], trn[:])

            def head_layer(out_ps, in_chunks, w_sb_, b_sb_, nslices, nk):
                for ns in range(nslices):
                    nc.tensor.matmul(out_ps[0:64, ns * 512:(ns + 1) * 512],
                                     on16_sb[0:1, 0:64],
                                     b_sb_[0:1, ns * 512:(ns + 1) * 512],
                                     start=True, stop=False)
                    for c in range(nk):
                        nc.tensor.matmul(out_ps[0:64, ns * 512:(ns + 1) * 512],
                                         in_chunks(c),
                                         w_sb_[:, c, ns * 512:(ns + 1) * 512],
                                         start=False, stop=(c == nk - 1))

            def transpose8(src_sb, tag):
                """[64, 1024] f16 -> [128, 8, 64] f16 chunks."""
                tr_a = pstr2.tile([128, 4, C], F16, tag="tr16")
                for j in range(4):
                    nc.tensor.transpose(tr_a[:, j, :],
                                        src_sb[0:64, j * 128:(j + 1) * 128],
                                        eye16_sb[0:64, 0:64])
                tr_b = pstr2.tile([128, 4, C], F16, tag="tr16")
                for j in range(4):
                    nc.tensor.transpose(tr_b[:, j, :],
                                        src_sb[0:64, 512 + j * 128:512 + (j + 1) * 128],
                                        eye16_sb[0:64, 0:64])
                dst = tailp.tile([128, 8, C], F16, tag=tag)
                nc.vector.tensor_copy(dst[:, 0:4, :], tr_a[:])
                nc.vector.tensor_copy(dst[:, 4:8, :], tr_b[:])
                return dst

            # uncertainty head: silu(u_in @ uW1.T + ub1) @ uW2.T + ub2
            u1_ps = pst.tile([128, H], F32, tag="pt")
            head_layer(u1_ps,
                       lambda c: lw_sb[:, c, :] if c < 4 else narrT[:, c - 4, :],
                       uw1_sb, ub1_sb, 2, 8)
            s1 = tailp.tile([128, H], F16, tag="s1")
            nc.scalar.activation(s1[0:64, :], u1_ps[0:64, :], AF.Silu)
            s1T = transpose8(s1, "s1T")
            u2_ps = pst.tile([128, H], F32, tag="pt")
            head_layer(u2_ps, lambda c: s1T[:, c, :], uw2_sb, ub2_sb, 2, 8)
            u2_sb = tailp.tile([128, H], F32, tag="u2")
            nc.vector.tensor_copy(u2_sb[0:64, :], u2_ps[0:64, :])
            nc.sync.dma_start(o_unc[:], u2_sb[0:64, :])

            # prediction head: silu(narr @ pW1.T + pb1) @ pW2.T + pb2
            p1_ps = pst.tile([128, H], F32, tag="pt")
            head_layer(p1_ps, lambda c: narrT[:, c, :], pw1_sb, pb1_sb, 2, 4)
            s2 = tailp.tile([128, H], F16, tag="s2")
            nc.scalar.activation(s2[0:64, :], p1_ps[0:64, :], AF.Silu)
            s2T = transpose8(s2, "s2T")
            p2_ps = pst.tile([128, BN], F32, tag="pq")
            head_layer(p2_ps, lambda c: s2T[:, c, :], pw2_sb, pb2_sb, 1, 8)
            p2_sb = tailp.tile([128, BN], F32, tag="p2")
            nc.vector.tensor_copy(p2_sb[0:64, :], p2_ps[0:64, :])
            nc.sync.dma_start(o_pred[:], p2_sb[0:64, :])

    nc.compile()
    return nc


def _chunk3(M):
    """[n*128, X] -> [128, n, X] contiguous."""
    n = M.shape[0] // 128
    return np.ascontiguousarray(M.reshape(n, 128, -1).transpose(1, 0, 2))


def _prep_shared(inputs):
    f16 = np.float16
    f32 = np.float32
    W_ih = np.asarray(inputs["W_ih"], f32)
    W_hh = np.asarray(inputs["W_hh"], f32)
    b_ih = np.asarray(inputs["b_ih"], f32)
    b_hh = np.asarray(inputs["b_hh"], f32)
    Wq = np.asarray(inputs["Wq"], f32)
    bq = np.asarray(inputs["bq"], f32)
    emb = np.asarray(inputs["emb"], f32)
    uW1 = np.asarray(inputs["uW1"], f32)
    uW2 = np.asarray(inputs["uW2"], f32)
    pW1 = np.asarray(inputs["pW1"], f32)
    pW2 = np.asarray(inputs["pW2"], f32)
    p = np.arange(128, dtype=f32)

    def bt(vec):
        # [1024] bias -> [128, 512] broadcast tile (col-split layout)
        out = np.empty((128, 512), f32)
        out[0:64, :] = vec[None, 0:512]
        out[64:128, :] = vec[None, 512:1024]
        return out

    brz = b_ih + b_hh
    wihT = W_ih.T  # [512, 3072]
    whhT = W_hh.T  # [1024, 3072]
    wih_hi = wihT.astype(f16)
    wih_lo = (wihT - wih_hi.astype(f32)).astype(f16)
    whh_hi = whhT.astype(f16)
    whh_lo = (whhT - whh_hi.astype(f32)).astype(f16)
    return {
        "wihT_hi": _chunk3(wih_hi),
        "wihT_lo": _chunk3(wih_lo),
        "whhT_hi": _chunk3(whh_hi),
        "whhT_lo": _chunk3(whh_lo),
        "bt_r": bt(brz[0:1024]),
        "bt_z": bt(brz[1024:2048]),
        "bt_nh": bt(b_hh[2048:3072]),
        "bt_ni": bt(b_ih[2048:3072]),
        "wqT": _chunk3(Wq.T).astype(f32),
        "bq_row": bq[None, :].astype(f32),
        "embT2": np.concatenate([2.0 * emb.T, 2.0 * emb.T], axis=0).astype(f32),
        "negnorm": (-(emb.astype(np.float64) ** 2).sum(1)
                    - 1e-9 * np.arange(NCODE))[None, :].astype(f32),
        "emb8": _chunk3(emb).astype(f32),
        "uw1T": _chunk3(uW1.T).astype(f16),
        "ub1_row": np.asarray(inputs["ub1"], f32)[None, :].astype(f16),
        "uw2T": _chunk3(uW2.T).astype(f16),
        "ub2_row": np.asarray(inputs["ub2"], f32)[None, :].astype(f16),
        "pw1T": _chunk3(pW1.T).astype(f16),
        "pb1_row": np.asarray(inputs["pb1"], f32)[None, :].astype(f16),
        "pw2T": _chunk3(pW2.T).astype(f16),
        "pb2_row": np.asarray(inputs["pb2"], f32)[None, :].astype(f16),
        "ones16": np.ones((1, 128), f16),
        "ones32": np.ones((1, 128), f32),
        "onescol": np.ones((128, 1), f32),
        "eye16": np.eye(128, dtype=f16),
        "eye32": np.eye(128, dtype=f32),
        "iota8": (p[:, None] + 128.0 * np.arange(8, dtype=f32)[None, :]).astype(f32),
    }


def kernel(**inputs):
    if "nc" not in _CACHE:
        _CACHE["nc"] = _build()
    nc = _CACHE["nc"]

    x = np.asarray(inputs["state_window"], np.float32)
    shared = _prep_shared(inputs)
    in_maps = []
    for ci in range(NCORES):
        shard = x[ci * BS:(ci + 1) * BS]          # [64, 64, 512]
        xt = shard.transpose(2, 1, 0)             # [512 d, 64 w, 64 b]
        m = dict(shared)
        xflat = np.ascontiguousarray(xt.reshape(D, W * BS))
        x_hi = xflat.astype(np.float16)
        x_lo = (xflat - x_hi.astype(np.float32)).astype(np.float16)
        m["xT_hi"] = _chunk3(x_hi)
        m["xT_lo"] = _chunk3(x_lo)
        m["lwT16"] = _chunk3(np.ascontiguousarray(shard[:, -1, :].T)).astype(np.float16)
        in_maps.append(m)

    res = run_bass_kernel_spmd(nc, in_maps, list(range(NCORES)))
    kernel.LAST_RESULT = res

    r = res.results
    code_indices = np.concatenate([r[c]["o_idx"] for c in range(NCORES)], axis=0)
    quantized_st = np.concatenate([r[c]["o_qst"] for c in range(NCORES)], axis=0)
    narrator = np.concatenate([r[c]["o_narr"] for c in range(NCORES)], axis=0)
    uncertainty = np.concatenate([r[c]["o_unc"] for c in range(NCORES)], axis=0)
    predicted = np.concatenate([r[c]["o_pred"] for c in range(NCORES)], axis=0)
    last_hidden = np.concatenate([r[c]["o_hid"] for c in range(NCORES)], axis=0)
    total = sum(float(r[c]["o_loss"][0, 0]) for c in range(NCORES))
    vq_loss = np.float32(1.25 * total / (B * K * C))
    return (code_indices.astype(np.int32), quantized_st, narrator, uncertainty,
            predicted, vq_loss, last_hidden)


# revision 19
# speedup vs baseline: 1.1142x; 1.1142x over previous
"""Trainium2 Bass kernel for nn_DiscreteNarrator (GRU + VQ codebook + heads).

Strategy: data-parallel over batch across 8 NeuronCores (64 rows/core).
Per core:
  - GRU recurrence runs fully on-chip. Layout: gh[b, g] with batch on
    PSUM partitions, gate slices on the free dim; dual column-group
    matmuls (tile_position) pack two independent 512-wide gate slices
    into one 128x128 PE pass so the 64-row batch uses the full array.
  - Matmul inputs in fp16 (10-bit mantissa; verified 0 argmin flips vs
    fp32 reference on the generated codebook margins), fp32 PSUM
    accumulation. The x-projection for step t+1 is fused into step t's
    PE stream (no separate gi_all phase, no DRAM staging).
  - VQ tail in fp32: queries, scores S = 2 q.e - |e|^2 - 1e-9*n (argmax
    == argmin-with-first-index-tie-break), one-hot matmul against the
    fp32 codebook so quantized rows are exact; heads use fp16 weights.
vq_loss partial sums are reduced on the host (the only cross-core term).
"""
import os
import sys
import types

import numpy as np

_HERE = os.path.dirname(os.path.abspath(__file__))


def _register_ntff_hook():
    """Make trace=True (BASS_TRACE=1) work under axon if the hook is absent."""
    try:
        from antenv.axon_hooks import get_axon_ntff_profile_hook  # noqa
        return
    except Exception:
        pass
    try:
        from trn_agent_boot.trn_boot import _ntff_profile_via_ctypes
        hook = _ntff_profile_via_ctypes("/opt/axon/libaxon_pjrt.so")
        mod = types.ModuleType("antenv.axon_hooks")
        mod.get_axon_ntff_profile_hook = lambda: hook
        mod.set_axon_ntff_profile_hook = lambda h: None
        import antenv
        sys.modules["antenv.axon_hooks"] = mod
        antenv.axon_hooks = mod
    except Exception:
        pass


_register_ntff_hook()

import concourse.tile as tile
import concourse.mybir as mybir
from concourse import bacc
from concourse.bass_utils import run_bass_kernel_spmd

F32 = mybir.dt.float32
F16 = mybir.dt.float16
U32 = mybir.dt.uint32
I32 = mybir.dt.int32
AF = mybir.ActivationFunctionType
OP = mybir.AluOpType

# problem dims (hardcoded per contract)
B, W, D, H = 512, 64, 512, 1024
NCODE, K, C = 1024, 8, 64
BN = K * C          # 512
G3 = 3 * H          # 3072
NCORES = 8
BS = B // NCORES    # 64

_CACHE = {}


def _hmap(c):
    """Free-dim offset of hT chunk c inside the [128, 512] hT tile."""
    return (c % 4) * 128 + (c // 4) * 64


def _build():
    from contextlib import ExitStack

    nc = bacc.Bacc(None, target_bir_lowering=False)

    # ---- inputs ----
    xT_hi = nc.dram_tensor("xT_hi", [128, 4, W * BS], F16, kind="ExternalInput")
    xT_lo = nc.dram_tensor("xT_lo", [128, 4, W * BS], F16, kind="ExternalInput")
    wihT_hi = nc.dram_tensor("wihT_hi", [128, 4, G3], F16, kind="ExternalInput")
    wihT_lo = nc.dram_tensor("wihT_lo", [128, 4, G3], F16, kind="ExternalInput")
    whhT_hi = nc.dram_tensor("whhT_hi", [128, 8, G3], F16, kind="ExternalInput")
    whhT_lo = nc.dram_tensor("whhT_lo", [128, 8, G3], F16, kind="ExternalInput")
    bt_r = nc.dram_tensor("bt_r", [128, 512], F32, kind="ExternalInput")
    bt_z = nc.dram_tensor("bt_z", [128, 512], F32, kind="ExternalInput")
    bt_nh = nc.dram_tensor("bt_nh", [128, 512], F32, kind="ExternalInput")
    bt_ni = nc.dram_tensor("bt_ni", [128, 512], F32, kind="ExternalInput")
    wqT = nc.dram_tensor("wqT", [128, 8, BN], F32, kind="ExternalInput")
    bq_row = nc.dram_tensor("bq_row", [1, BN], F32, kind="ExternalInput")
    embT2 = nc.dram_tensor("embT2", [128, NCODE], F32, kind="ExternalInput")
    negnorm = nc.dram_tensor("negnorm", [1, NCODE], F32, kind="ExternalInput")
    emb8 = nc.dram_tensor("emb8", [128, 8, C], F32, kind="ExternalInput")
    lwT16 = nc.dram_tensor("lwT16", [128, 4, BS], F16, kind="ExternalInput")
    uw1T = nc.dram_tensor("uw1T", [128, 8, H], F16, kind="ExternalInput")
    ub1_row = nc.dram_tensor("ub1_row", [1, H], F16, kind="ExternalInput")
    uw2T = nc.dram_tensor("uw2T", [128, 8, H], F16, kind="ExternalInput")
    ub2_row = nc.dram_tensor("ub2_row", [1, H], F16, kind="ExternalInput")
    pw1T = nc.dram_tensor("pw1T", [128, 4, H], F16, kind="ExternalInput")
    pb1_row = nc.dram_tensor("pb1_row", [1, H], F16, kind="ExternalInput")
    pw2T = nc.dram_tensor("pw2T", [128, 8, BN], F16, kind="ExternalInput")
    pb2_row = nc.dram_tensor("pb2_row", [1, BN], F16, kind="ExternalInput")
    ones16 = nc.dram_tensor("ones16", [1, 128], F16, kind="ExternalInput")
    ones32 = nc.dram_tensor("ones32", [1, 128], F32, kind="ExternalInput")
    onescol = nc.dram_tensor("onescol", [128, 1], F32, kind="ExternalInput")
    eye16 = nc.dram_tensor("eye16", [128, 128], F16, kind="ExternalInput")
    eye32 = nc.dram_tensor("eye32", [128, 128], F32, kind="ExternalInput")
    iota8 = nc.dram_tensor("iota8", [128, 8], F32, kind="ExternalInput")

    # ---- outputs ----
    o_idx = nc.dram_tensor("o_idx", [BS, K], I32, kind="ExternalOutput")
    o_qst = nc.dram_tensor("o_qst", [BS, K, C], F32, kind="ExternalOutput")
    o_narr = nc.dram_tensor("o_narr", [BS, BN], F32, kind="ExternalOutput")
    o_unc = nc.dram_tensor("o_unc", [BS, H], F32, kind="ExternalOutput")
    o_pred = nc.dram_tensor("o_pred", [BS, BN], F32, kind="ExternalOutput")
    o_loss = nc.dram_tensor("o_loss", [1, 1], F32, kind="ExternalOutput")
    o_hid = nc.dram_tensor("o_hid", [BS, H], F32, kind="ExternalOutput")

    with tile.TileContext(nc) as tc, ExitStack() as stack:
        cpool = stack.enter_context(tc.tile_pool(name="cpool", bufs=1))
        dyn2 = stack.enter_context(tc.tile_pool(name="dyn2", bufs=2))
        dyng = stack.enter_context(tc.tile_pool(name="dyng", bufs=8))
        dynx = stack.enter_context(tc.tile_pool(name="dynx", bufs=3))

        # consts
        on16_sb = cpool.tile([1, 128], F16, tag="on16")
        on32_sb = cpool.tile([1, 128], F32, tag="on32")
        onc_sb = cpool.tile([128, 1], F32, tag="onc")
        eye16_sb = cpool.tile([128, 128], F16, tag="eye16")
        eye32_sb = cpool.tile([128, 128], F32, tag="eye32")
        iota_sb = cpool.tile([128, 8], F32, tag="iota")
        btr_sb = cpool.tile([128, 512], F32, tag="btr")
        btz_sb = cpool.tile([128, 512], F32, tag="btz")
        btnh_sb = cpool.tile([128, 512], F32, tag="btnh")
        btni_sb = cpool.tile([128, 512], F32, tag="btni")
        zrow16 = cpool.tile([1, 512], F16, tag="zrow16")
        nc.vector.memset(zrow16[:], 0.0)
        zrow32 = cpool.tile([1, 512], F32, tag="zrow32")
        nc.vector.memset(zrow32[:], 0.0)
        for dst, src in ((on16_sb, ones16), (on32_sb, ones32), (onc_sb, onescol),
                         (eye16_sb, eye16), (eye32_sb, eye32), (iota_sb, iota8),
                         (btr_sb, bt_r), (btz_sb, bt_z), (btnh_sb, bt_nh),
                         (btni_sb, bt_ni)):
            nc.sync.dma_start(dst[:], src[:])

        # ---------------- recurrence ----------------
        h2_prev = dyn2.tile([128, 512], F32, tag="h2")
        nc.vector.memset(h2_prev[:], 0.0)

        xt_tiles = {}

        def prefetch_xt(t):
            if t > W:
                return
            xh = dynx.tile([128, 4, BS], F16, tag="xth")
            nc.sync.dma_start(xh[:], xT_hi[:, :, (t - 1) * BS:t * BS])
            xl = dynx.tile([128, 4, BS], F16, tag="xtl")
            nc.sync.dma_start(xl[:], xT_lo[:, :, (t - 1) * BS:t * BS])
            xt_tiles[t] = (xh, xl)

        with tc.tile_pool(name="rw", bufs=1) as rw:
            wih_hi = rw.tile([128, 4, G3], F16, tag="wihhi")
            wih_lo = rw.tile([128, 4, G3], F16, tag="wihlo")
            whh_hi = rw.tile([128, 8, G3], F16, tag="whhhi")
            whh_lo = rw.tile([128, 8, G3], F16, tag="whhlo")
            nc.sync.dma_start(wih_hi[:], wihT_hi[:])
            nc.sync.dma_start(wih_lo[:], wihT_lo[:])
            nc.sync.dma_start(whh_hi[:], whhT_hi[:])
            nc.sync.dma_start(whh_lo[:], whhT_lo[:])

            with tc.tile_pool(name="ghp", bufs=2, space="PSUM") as ghp, \
                 tc.tile_pool(name="smp", bufs=2, space="PSUM") as smp:

                def a_block(t, first_step):
                    """x-projection matmuls (3-term hi/lo) for step t."""
                    xh, xl = xt_tiles.pop(t)
                    gh = ghp.tile([128, 1536], F32, tag="gh")
                    i_n = smp.tile([128, 512], F32, tag="sm")
                    o128 = on16_sb[0:1, 0:128]
                    # one start=True zeroing matmul per bank region
                    nc.tensor.matmul(gh[:, 0:512], o128, zrow16[0:1, :],
                                     start=True, stop=False, skip_group_check=True)
                    nc.tensor.matmul(gh[:, 512:1024], o128, zrow16[0:1, :],
                                     start=True, stop=False, skip_group_check=True)
                    nc.tensor.matmul(gh[:, 1024:1536], o128, zrow16[0:1, :],
                                     start=True, stop=first_step, skip_group_check=True)
                    nc.tensor.matmul(i_n[:, :], o128, zrow16[0:1, :],
                                     start=True, stop=False, skip_group_check=True)
                    terms = ((xh, wih_hi), (xh, wih_lo), (xl, wih_hi))
                    for col in range(2):
                        po = col * 64
                        cs = col * 512
                        for ti, (xa, wa) in enumerate(terms):
                            for c in range(4):
                                last = (ti == 2 and c == 3)
                                nc.tensor.matmul(
                                    gh[po:po + 64, 0:512], xa[:, c, :],
                                    wa[:, c, cs:cs + 512],
                                    start=False, stop=(last and first_step),
                                    skip_group_check=True,
                                    tile_position=(0, po))
                                nc.tensor.matmul(
                                    gh[po:po + 64, 512:1024], xa[:, c, :],
                                    wa[:, c, 1024 + cs:1024 + cs + 512],
                                    start=False, stop=(last and first_step),
                                    skip_group_check=True,
                                    tile_position=(0, po))
                                nc.tensor.matmul(
                                    i_n[po:po + 64, :], xa[:, c, :],
                                    wa[:, c, 2048 + cs:2048 + cs + 512],
                                    start=False, stop=last,
                                    skip_group_check=True,
                                    tile_position=(0, po))
                    return gh, i_n

                def b_block(gh, hT_prev):
                    """h-recurrence matmuls (3-term hi/lo). Region order: r, hn, z."""
                    def hchunk(c, lo):
                        off = 512 * lo + _hmap(c)
                        return hT_prev[:, off:off + 64]
                    for g0, fr in ((0, 0), (2048, 1024), (1024, 512)):
                        for ti in range(3):
                            wa = whh_lo if ti == 1 else whh_hi
                            hlo = 1 if ti == 2 else 0
                            for c in range(8):
                                hc = hchunk(c, hlo)
                                for col in range(2):
                                    po = col * 64
                                    nc.tensor.matmul(
                                        gh[po:po + 64, fr:fr + 512], hc,
                                        wa[:, c, g0 + col * 512:g0 + col * 512 + 512],
                                        start=False,
                                        stop=(ti == 2 and c == 7),
                                        skip_group_check=True,
                                        tile_position=(0, po))

                def c_block(gh, i_n, h2_in, t):
                    last = (t == W)
                    r_pre = dyng.tile([128, 512], F32, tag="gate")
                    nc.vector.tensor_tensor(r_pre[:], gh[:, 0:512], btr_sb[:], OP.add)
                    r_sb = dyng.tile([128, 512], F32, tag="gate")
                    nc.scalar.activation(r_sb[:], r_pre[:], AF.Sigmoid)
                    t2a = dyng.tile([128, 512], F32, tag="gate")
                    nc.vector.tensor_tensor(t2a[:], i_n[:], btni_sb[:], OP.add)
                    t1 = dyng.tile([128, 512], F32, tag="gate")
                    if t == 1:
                        nc.vector.tensor_tensor(t1[:], btnh_sb[:], r_sb[:], OP.mult)
                    else:
                        t0 = dyng.tile([128, 512], F32, tag="gate")
                        nc.vector.tensor_tensor(t0[:], gh[:, 1024:1536], btnh_sb[:], OP.add)
                        nc.vector.tensor_tensor(t1[:], t0[:], r_sb[:], OP.mult)
                    t2 = dyng.tile([128, 512], F32, tag="gate")
                    nc.vector.tensor_tensor(t2[:], t1[:], t2a[:], OP.add)
                    n_sb = dyng.tile([128, 512], F32, tag="gate")
                    nc.scalar.activation(n_sb[:], t2[:], AF.Tanh)
                    z_pre = dyng.tile([128, 512], F32, tag="gate")
                    nc.vector.tensor_tensor(z_pre[:], gh[:, 512:1024], btz_sb[:], OP.add)
                    z_sb = dyng.tile([128, 512], F32, tag="gate")
                    nc.scalar.activation(z_sb[:], z_pre[:], AF.Sigmoid)
                    v_sb = dyng.tile([128, 512], F32, tag="gate")
                    nc.scalar.activation(v_sb[:], z_pre[:], AF.Sigmoid, scale=-1.0)
                    u_sb = dyng.tile([128, 512], F32, tag="gate")
                    nc.vector.tensor_tensor(u_sb[:], z_sb[:], h2_in[:], OP.mult)
                    w_sb = dyng.tile([128, 512], F32, tag="gate")
                    nc.vector.tensor_tensor(w_sb[:], v_sb[:], n_sb[:], OP.mult)
                    h2_t = dyn2.tile([128, 512], F32, tag="h2")
                    nc.vector.tensor_tensor(h2_t[:], u_sb[:], w_sb[:], OP.add)
                    if last:
                        return h2_t, None, None
                    h2b_hi = dyn2.tile([128, 512], F16, tag="h2bh")
                    nc.scalar.copy(h2b_hi[:], h2_t[:])
                    h2b_lo = dyn2.tile([128, 512], F16, tag="h2bl")
                    nc.vector.tensor_tensor(h2b_lo[:], h2_t[:], h2b_hi[:], OP.subtract)
                    return h2_t, h2b_hi, h2b_lo

                def d_block(h2b_hi, h2b_lo):
                    tr = smp.tile([128, 1024], F16, tag="sm")
                    for j in range(4):
                        nc.tensor.transpose(tr[:, j * 128:(j + 1) * 128],
                                            h2b_hi[:, j * 128:(j + 1) * 128], eye16_sb[:])
                    for j in range(4):
                        nc.tensor.transpose(tr[:, 512 + j * 128:512 + (j + 1) * 128],
                                            h2b_lo[:, j * 128:(j + 1) * 128], eye16_sb[:])
                    hT_t = dyn2.tile([128, 1024], F16, tag="hT")
                    nc.vector.tensor_copy(hT_t[:], tr[:])
                    return hT_t

                prefetch_xt(1)
                prefetch_xt(2)
                gh_t, in_t = a_block(1, True)
                hT_prev = None
                for t in range(1, W + 1):
                    prefetch_xt(t + 2)
                    if t > 1:
                        b_block(gh_t, hT_prev)
                    h2_t, h2b_hi, h2b_lo = c_block(gh_t, in_t, h2_prev, t)
                    if t < W:
                        gh_t, in_t = a_block(t + 1, False)
                        hT_prev = d_block(h2b_hi, h2b_lo)
                    h2_prev = h2_t

        h2f = h2_prev  # [128, 512] f32: p<64 -> h[b, 0:512]; p>=64 -> h[b, 512:1024]
        nc.sync.dma_start(o_hid[:, 0:512], h2f[0:64, :])
        nc.sync.dma_start(o_hid[:, 512:1024], h2f[64:128, :])

        # tail weights (allocated after the recurrence weights free their space)
        tailp = stack.enter_context(tc.tile_pool(name="tailp", bufs=1))
        wq_sb = tailp.tile([128, 8, BN], F32, tag="wq")
        uw1_sb = tailp.tile([128, 8, H], F16, tag="uw1")
        embT2_sb = tailp.tile([128, NCODE], F32, tag="embT2")
        nn_sb = tailp.tile([1, NCODE], F32, tag="negnorm")
        emb8_sb = tailp.tile([128, 8, C], F32, tag="emb8")
        lw_sb = tailp.tile([128, 4, BS], F16, tag="lw")
        uw2_sb = tailp.tile([128, 8, H], F16, tag="uw2")
        pw1_sb = tailp.tile([128, 4, H], F16, tag="pw1")
        pw2_sb = tailp.tile([128, 8, BN], F16, tag="pw2")
        bq_sb = tailp.tile([1, BN], F32, tag="bq")
        ub1_sb = tailp.tile([1, H], F16, tag="ub1")
        ub2_sb = tailp.tile([1, H], F16, tag="ub2")
        pb1_sb = tailp.tile([1, H], F16, tag="pb1")
        pb2_sb = tailp.tile([1, BN], F16, tag="pb2")
        for dst, src in ((wq_sb, wqT), (uw1_sb, uw1T), (embT2_sb, embT2),
                         (nn_sb, negnorm), (emb8_sb, emb8), (lw_sb, lwT16),
                         (uw2_sb, uw2T), (pw1_sb, pw1T), (pw2_sb, pw2T),
                         (bq_sb, bq_row), (ub1_sb, ub1_row), (ub2_sb, ub2_row),
                         (pb1_sb, pb1_row), (pb2_sb, pb2_row)):
            nc.sync.dma_start(dst[:], src[:])

        # ---------------- tail ----------------
        with tc.tile_pool(name="pstail", bufs=1, space="PSUM") as pst, \
             tc.tile_pool(name="pstr1", bufs=1, space="PSUM") as pstr1, \
             tc.tile_pool(name="pstr2", bufs=2, space="PSUM") as pstr2, \
             tc.tile_pool(name="dscr", bufs=1, space="DRAM") as dscr:

            # fp32 transposed final hidden
            trh = pstr1.tile([128, 512], F32, tag="tr")
            for j in range(4):
                nc.tensor.transpose(trh[:, j * 128:(j + 1) * 128],
                                    h2f[:, j * 128:(j + 1) * 128], eye32_sb[:])
            hT32 = tailp.tile([128, 512], F32, tag="hT32")
            nc.vector.tensor_copy(hT32[:], trh[:])

            # queries q = h @ Wq.T + bq  [64, 512]
            q_ps = pst.tile([128, BN], F32, tag="pq")
            nc.tensor.matmul(q_ps[0:64, :], on32_sb[0:1, 0:64], bq_sb[0:1, :],
                             start=True, stop=False)
            for c in range(8):
                nc.tensor.matmul(q_ps[0:64, :], hT32[:, _hmap(c):_hmap(c) + 64],
                                 wq_sb[:, c, :], start=False, stop=(c == 7))
            q_sb = tailp.tile([128, BN], F32, tag="q")
            nc.vector.tensor_copy(q_sb[0:64, :], q_ps[0:64, :])

            # qT chunks: [128, 4, 64], chunk j partitions = (k=2j: c | k=2j+1: c)
            trq = pstr1.tile([128, 4, C], F32, tag="trq")
            for j in range(4):
                nc.tensor.transpose(trq[:, j, :], q_sb[0:64, j * 128:(j + 1) * 128],
                                    eye32_sb[0:64, 0:64])
            qT_sb = tailp.tile([128, 4, C], F32, tag="qT")
            nc.vector.tensor_copy(qT_sb[:], trq[:])

            # scores S_j = 2 q.e - |e|^2 - 1e-9 n  [128 (2 k), 1024 codes]
            idx_bounce = dscr.tile([1, 512], F32, tag="idxb")
            for j in range(4):
                S_ps = pst.tile([128, NCODE], F32, tag="pt")
                for ns in range(2):
                    nc.tensor.matmul(
                        S_ps[:, ns * 512:(ns + 1) * 512],
                        on32_sb[0:1, 0:128],
                        nn_sb[0:1, ns * 512:(ns + 1) * 512],
                        start=True, stop=False, skip_group_check=True)
                    for col in range(2):
                        po = col * 64
                        nc.tensor.matmul(
                            S_ps[po:po + 64, ns * 512:(ns + 1) * 512],
                            qT_sb[po:po + 64, j, :],
                            embT2_sb[po:po + 64, ns * 512:(ns + 1) * 512],
                            start=False, stop=True, skip_group_check=True,
                            tile_position=(po, po))
                S_sb = tailp.tile([128, NCODE], F32, tag="S")
                nc.vector.tensor_copy(S_sb[:], S_ps[:])
                mx = tailp.tile([128, 8], F32, tag="mx")
                mi = tailp.tile([128, 8], U32, tag="mi")
                nc.vector.max_with_indices(mx[:], mi[:], S_sb[:])
                nc.sync.dma_start(o_idx[:, 2 * j:2 * j + 1], mi[0:64, 0:1].bitcast(I32))
                nc.sync.dma_start(o_idx[:, 2 * j + 1:2 * j + 2], mi[64:128, 0:1].bitcast(I32))
                idxf = tailp.tile([128, 1], F32, tag="idxf")
                nc.vector.tensor_copy(idxf[:], mi[:, 0:1])
                nc.sync.dma_start(idx_bounce[0:1, j * 128:(j + 1) * 128], idxf[:, 0:1])

            idxrow = tailp.tile([1, 512], F32, tag="idxrow")
            nc.sync.dma_start(idxrow[:], idx_bounce[:])

            # broadcast idx over partitions; q index order along free = (k, b)
            bc_ps = pst.tile([128, 512], F32, tag="pq")
            nc.tensor.matmul(bc_ps[:], on32_sb[0:1, :], idxrow[0:1, :],
                             start=True, stop=True)
            bc_sb = dyng.tile([128, 512], F32, tag="gate")
            nc.vector.tensor_copy(bc_sb[:], bc_ps[:])

            # one-hot matmul -> narr [64, (k c)] (exact fp32 emb rows)
            narr_ps = pst.tile([128, BN], F32, tag="pq")
            nc.tensor.matmul(narr_ps[0:64, :], on32_sb[0:1, 0:64], zrow32[0:1, :],
                             start=True, stop=False)
            for nci in range(8):
                oh = dyng.tile([128, 512], F32, tag="gate")
                nc.vector.tensor_scalar(oh[:], bc_sb[:], iota_sb[:, nci:nci + 1], None,
                                        op0=OP.is_equal)
                for k in range(8):
                    nc.tensor.matmul(narr_ps[0:64, k * C:(k + 1) * C],
                                     oh[:, k * 64:(k + 1) * 64], emb8_sb[:, nci, :],
                                     start=False, stop=(nci == 7 and k == 7))
            narr_sb = tailp.tile([128, BN], F32, tag="narr")
            nc.vector.tensor_copy(narr_sb[0:64, :], narr_ps[0:64, :])
            nc.sync.dma_start(o_qst.rearrange("b k c -> b (k c)"), narr_sb[0:64, :])
            nc.sync.dma_start(o_narr[:], narr_sb[0:64, :])

            # vq loss partial: sum((q - narr)^2)
            d_sb = dyng.tile([128, BN], F32, tag="gate")
            nc.vector.tensor_tensor(d_sb[0:64, :], q_sb[0:64, :], narr_sb[0:64, :],
                                    OP.subtract)
            d2 = dyng.tile([128, BN], F32, tag="gate")
            dcol = tailp.tile([128, 1], F32, tag="dcol")
            nc.vector.scalar_tensor_tensor(d2[0:64, :], d_sb[0:64, :], 0.0,
                                           d_sb[0:64, :], op0=OP.add, op1=OP.mult,
                                           accum_out=dcol[0:64, :])
            loss_ps = pst.tile([128, 512], F32, tag="pq")
            nc.tensor.matmul(loss_ps[0:1, 0:1], dcol[0:64, 0:1], onc_sb[0:64, 0:1],
                             start=True, stop=True)
            loss_sb = tailp.tile([1, 1], F32, tag="loss")
            nc.vector.tensor_copy(loss_sb[:], loss_ps[0:1, 0:1])
            nc.sync.dma_start(o_loss[:], loss_sb[:])

            # narrT fp16 chunks for the heads
            narr16 = tailp.tile([128, BN], F16, tag="narr16")
            nc.scalar.copy(narr16[0:64, :], narr_sb[0:64, :])
            trn = pstr2.tile([128, 4, C], F16, tag="tr16")
            for j in range(4):
                nc.tensor.transpose(trn[:, j, :], narr16[0:64, j * 128:(j + 1) * 128],
                                    eye16_sb[0:64, 0:64])
            narrT = tailp.tile([128, 4, C], F16, tag="narrT")
            nc.vector.tensor_copy(narrT[:], trn[:])

            def head_layer(out_ps, in_chunks, w_sb_, b_sb_, nslices, nk):
                for ns in range(nslices):
                    nc.tensor.matmul(out_ps[0:64, ns * 512:(ns + 1) * 512],
                                     on16_sb[0:1, 0:64],
                                     b_sb_[0:1, ns * 512:(ns + 1) * 512],
                                     start=True, stop=False)
                    for c in range(nk):
                        nc.tensor.matmul(out_ps[0:64, ns * 512:(ns + 1) * 512],
                                         in_chunks(c),
                                         w_sb_[:, c, ns * 512:(ns + 1) * 512],
                                         start=False, stop=(c == nk - 1))

            def transpose8(src_sb, tag):
                """[64, 1024] f16 -> [128, 8, 64] f16 chunks."""
                tr_a = pstr2.tile([128, 4, C], F16, tag="tr16")
                for j in range(4):
                    nc.tensor.transpose(tr_a[:, j, :],
                                        src_sb[0:64, j * 128:(j + 1) * 128],
                                        eye16_sb[0:64, 0:64])
                tr_b = pstr2.tile([128, 4, C], F16, tag="tr16")
                for j in range(4):
                    nc.tensor.transpose(tr_b[:, j, :],
                                        src_sb[0:64, 512 + j * 128:512 + (j + 1) * 128],
                                        eye16_sb[0:64, 0:64])
                dst = tailp.tile([128, 8, C], F16, tag=tag)
                nc.vector.tensor_copy(dst[:, 0:4, :], tr_a[:])
                nc.vector.tensor_copy(dst[:, 4:8, :], tr_b[:])
                return dst

            # uncertainty head: silu(u_in @ uW1.T + ub1) @ uW2.T + ub2
            u1_ps = pst.tile([128, H], F32, tag="pt")
            head_layer(u1_ps,
                       lambda c: lw_sb[:, c, :] if c < 4 else narrT[:, c - 4, :],
                       uw1_sb, ub1_sb, 2, 8)
            s1 = tailp.tile([128, H], F16, tag="s1")
            nc.scalar.activation(s1[0:64, :], u1_ps[0:64, :], AF.Silu)
            s1T = transpose8(s1, "s1T")
            u2_ps = pst.tile([128, H], F32, tag="pt")
            head_layer(u2_ps, lambda c: s1T[:, c, :], uw2_sb, ub2_sb, 2, 8)
            u2_sb = tailp.tile([128, H], F32, tag="u2")
            nc.vector.tensor_copy(u2_sb[0:64, :], u2_ps[0:64, :])
            nc.sync.dma_start(o_unc[:], u2_sb[0:64, :])

            # prediction head: silu(narr @ pW1.T + pb1) @ pW2.T + pb2
            p1_ps = pst.tile([128, H], F32, tag="pt")
            head_layer(p1_ps, lambda c: narrT[:, c, :], pw1_sb, pb1_sb, 2, 4)
            s2 = tailp.tile([128, H], F16, tag="s2")
            nc.scalar.activation(s2[0:64, :], p1_ps[0:64, :], AF.Silu)
            s2T = transpose8(s2, "s2T")
            p2_ps = pst.tile([128, BN], F32, tag="pq")
            head_layer(p2_ps, lambda c: s2T[:, c, :], pw2_sb, pb2_sb, 1, 8)
            p2_sb = tailp.tile([128, BN], F32, tag="p2")
            nc.vector.tensor_copy(p2_sb[0:64, :], p2_ps[0:64, :])
            nc.sync.dma_start(o_pred[:], p2_sb[0:64, :])

    nc.compile()
    return nc


def _chunk3(M):
    """[n*128, X] -> [128, n, X] contiguous."""
    n = M.shape[0] // 128
    return np.ascontiguousarray(M.reshape(n, 128, -1).transpose(1, 0, 2))


def _prep_shared(inputs):
    f16 = np.float16
    f32 = np.float32
    W_ih = np.asarray(inputs["W_ih"], f32)
    W_hh = np.asarray(inputs["W_hh"], f32)
    b_ih = np.asarray(inputs["b_ih"], f32)
    b_hh = np.asarray(inputs["b_hh"], f32)
    Wq = np.asarray(inputs["Wq"], f32)
    bq = np.asarray(inputs["bq"], f32)
    emb = np.asarray(inputs["emb"], f32)
    uW1 = np.asarray(inputs["uW1"], f32)
    uW2 = np.asarray(inputs["uW2"], f32)
    pW1 = np.asarray(inputs["pW1"], f32)
    pW2 = np.asarray(inputs["pW2"], f32)
    p = np.arange(128, dtype=f32)

    def bt(vec):
        # [1024] bias -> [128, 512] broadcast tile (col-split layout)
        out = np.empty((128, 512), f32)
        out[0:64, :] = vec[None, 0:512]
        out[64:128, :] = vec[None, 512:1024]
        return out

    brz = b_ih + b_hh
    wihT = W_ih.T  # [512, 3072]
    whhT = W_hh.T  # [1024, 3072]
    wih_hi = wihT.astype(f16)
    wih_lo = (wihT - wih_hi.astype(f32)).astype(f16)
    whh_hi = whhT.astype(f16)
    whh_lo = (whhT - whh_hi.astype(f32)).astype(f16)
    return {
        "wihT_hi": _chunk3(wih_hi),
        "wihT_lo": _chunk3(wih_lo),
        "whhT_hi": _chunk3(whh_hi),
        "whhT_lo": _chunk3(whh_lo),
        "bt_r": bt(brz[0:1024]),
        "bt_z": bt(brz[1024:2048]),
        "bt_nh": bt(b_hh[2048:3072]),
        "bt_ni": bt(b_ih[2048:3072]),
        "wqT": _chunk3(Wq.T).astype(f32),
        "bq_row": bq[None, :].astype(f32),
        "embT2": np.concatenate([2.0 * emb.T, 2.0 * emb.T], axis=0).astype(f32),
        "negnorm": (-(emb.astype(np.float64) ** 2).sum(1)
                    - 1e-9 * np.arange(NCODE))[None, :].astype(f32),
        "emb8": _chunk3(emb).astype(f32),
        "uw1T": _chunk3(uW1.T).astype(f16),
        "ub1_row": np.asarray(inputs["ub1"], f32)[None, :].astype(f16),
        "uw2T": _chunk3(uW2.T).astype(f16),
        "ub2_row": np.asarray(inputs["ub2"], f32)[None, :].astype(f16),
        "pw1T": _chunk3(pW1.T).astype(f16),
        "pb1_row": np.asarray(inputs["pb1"], f32)[None, :].astype(f16),
        "pw2T": _chunk3(pW2.T).astype(f16),
        "pb2_row": np.asarray(inputs["pb2"], f32)[None, :].astype(f16),
        "ones16": np.ones((1, 128), f16),
        "ones32": np.ones((1, 128), f32),
        "onescol": np.ones((128, 1), f32),
        "eye16": np.eye(128, dtype=f16),
        "eye32": np.eye(128, dtype=f32),
        "iota8": (p[:, None] + 128.0 * np.arange(8, dtype=f32)[None, :]).astype(f32),
    }


def kernel(**inputs):
    if "nc" not in _CACHE:
        _CACHE["nc"] = _build()
    nc = _CACHE["nc"]

    x = np.asarray(inputs["state_window"], np.float32)
    shared = _prep_shared(inputs)
    in_maps = []
    for ci in range(NCORES):
        shard = x[ci * BS:(ci + 1) * BS]          # [64, 64, 512]
        xt = shard.transpose(2, 1, 0)             # [512 d, 64 w, 64 b]
        m = dict(shared)
        xflat = np.ascontiguousarray(xt.reshape(D, W * BS))
        x_hi = xflat.astype(np.float16)
        x_lo = (xflat - x_hi.astype(np.float32)).astype(np.float16)
        m["xT_hi"] = _chunk3(x_hi)
        m["xT_lo"] = _chunk3(x_lo)
        m["lwT16"] = _chunk3(np.ascontiguousarray(shard[:, -1, :].T)).astype(np.float16)
        in_maps.append(m)

    res = run_bass_kernel_spmd(nc, in_maps, list(range(NCORES)))
    kernel.LAST_RESULT = res

    r = res.results
    code_indices = np.concatenate([r[c]["o_idx"] for c in range(NCORES)], axis=0)
    quantized_st = np.concatenate([r[c]["o_qst"] for c in range(NCORES)], axis=0)
    narrator = np.concatenate([r[c]["o_narr"] for c in range(NCORES)], axis=0)
    uncertainty = np.concatenate([r[c]["o_unc"] for c in range(NCORES)], axis=0)
    predicted = np.concatenate([r[c]["o_pred"] for c in range(NCORES)], axis=0)
    last_hidden = np.concatenate([r[c]["o_hid"] for c in range(NCORES)], axis=0)
    total = sum(float(r[c]["o_loss"][0, 0]) for c in range(NCORES))
    vq_loss = np.float32(1.25 * total / (B * K * C))
    return (code_indices.astype(np.int32), quantized_st, narrator, uncertainty,
            predicted, vq_loss, last_hidden)


# revision 20
# speedup vs baseline: 1.3662x; 1.2262x over previous
"""Trainium2 Bass kernel for nn_DiscreteNarrator (GRU + VQ codebook + heads).

Strategy: data-parallel over batch across 8 NeuronCores (64 rows/core).
Per core:
  - GRU recurrence runs fully on-chip. Layout: gh[b, g] with batch on
    PSUM partitions, gate slices on the free dim; dual column-group
    matmuls (tile_position) pack two independent 512-wide gate slices
    into one 128x128 PE pass so the 64-row batch uses the full array.
  - Matmul inputs in fp16 (10-bit mantissa; verified 0 argmin flips vs
    fp32 reference on the generated codebook margins), fp32 PSUM
    accumulation. The x-projection for step t+1 is fused into step t's
    PE stream (no separate gi_all phase, no DRAM staging).
  - VQ tail in fp32: queries, scores S = 2 q.e - |e|^2 - 1e-9*n (argmax
    == argmin-with-first-index-tie-break), one-hot matmul against the
    fp32 codebook so quantized rows are exact; heads use fp16 weights.
vq_loss partial sums are reduced on the host (the only cross-core term).
"""
import os
import sys
import types

import numpy as np

_HERE = os.path.dirname(os.path.abspath(__file__))


def _register_ntff_hook():
    """Make trace=True (BASS_TRACE=1) work under axon if the hook is absent."""
    try:
        from antenv.axon_hooks import get_axon_ntff_profile_hook  # noqa
        return
    except Exception:
        pass
    try:
        from trn_agent_boot.trn_boot import _ntff_profile_via_ctypes
        hook = _ntff_profile_via_ctypes("/opt/axon/libaxon_pjrt.so")
        mod = types.ModuleType("antenv.axon_hooks")
        mod.get_axon_ntff_profile_hook = lambda: hook
        mod.set_axon_ntff_profile_hook = lambda h: None
        import antenv
        sys.modules["antenv.axon_hooks"] = mod
        antenv.axon_hooks = mod
    except Exception:
        pass


_register_ntff_hook()

import concourse.tile as tile
import concourse.mybir as mybir
from concourse import bacc
from concourse.bass_utils import run_bass_kernel_spmd

F32 = mybir.dt.float32
F16 = mybir.dt.float16
U32 = mybir.dt.uint32
I32 = mybir.dt.int32
AF = mybir.ActivationFunctionType
OP = mybir.AluOpType

# problem dims (hardcoded per contract)
B, W, D, H = 512, 64, 512, 1024
NCODE, K, C = 1024, 8, 64
BN = K * C          # 512
G3 = 3 * H          # 3072
NCORES = 8
BS = B // NCORES    # 64

_CACHE = {}


def _hmap(c):
    """Free-dim offset of hT chunk c inside the [128, 512] hT tile."""
    return (c % 4) * 128 + (c // 4) * 64


def _build():
    from contextlib import ExitStack

    nc = bacc.Bacc(None, target_bir_lowering=False)

    # ---- inputs ----
    xT_hi = nc.dram_tensor("xT_hi", [128, 4, W * BS], F16, kind="ExternalInput")
    xT_lo = nc.dram_tensor("xT_lo", [128, 4, W * BS], F16, kind="ExternalInput")
    wihT_hi = nc.dram_tensor("wihT_hi", [128, 4, G3], F16, kind="ExternalInput")
    wihT_lo = nc.dram_tensor("wihT_lo", [128, 4, G3], F16, kind="ExternalInput")
    whhT_hi = nc.dram_tensor("whhT_hi", [128, 8, G3], F16, kind="ExternalInput")
    whhT_lo = nc.dram_tensor("whhT_lo", [128, 8, G3], F16, kind="ExternalInput")
    bt_r = nc.dram_tensor("bt_r", [128, 512], F32, kind="ExternalInput")
    bt_z = nc.dram_tensor("bt_z", [128, 512], F32, kind="ExternalInput")
    bt_nh = nc.dram_tensor("bt_nh", [128, 512], F32, kind="ExternalInput")
    bt_ni = nc.dram_tensor("bt_ni", [128, 512], F32, kind="ExternalInput")
    wqT = nc.dram_tensor("wqT", [128, 8, BN], F32, kind="ExternalInput")
    bq_row = nc.dram_tensor("bq_row", [1, BN], F32, kind="ExternalInput")
    embT2 = nc.dram_tensor("embT2", [128, NCODE], F32, kind="ExternalInput")
    negnorm = nc.dram_tensor("negnorm", [1, NCODE], F32, kind="ExternalInput")
    emb8 = nc.dram_tensor("emb8", [128, 8, C], F32, kind="ExternalInput")
    lwT16 = nc.dram_tensor("lwT16", [128, 4, BS], F16, kind="ExternalInput")
    uw1T = nc.dram_tensor("uw1T", [128, 8, H], F16, kind="ExternalInput")
    ub1_row = nc.dram_tensor("ub1_row", [1, H], F16, kind="ExternalInput")
    uw2T = nc.dram_tensor("uw2T", [128, 8, H], F16, kind="ExternalInput")
    ub2_row = nc.dram_tensor("ub2_row", [1, H], F16, kind="ExternalInput")
    pw1T = nc.dram_tensor("pw1T", [128, 4, H], F16, kind="ExternalInput")
    pb1_row = nc.dram_tensor("pb1_row", [1, H], F16, kind="ExternalInput")
    pw2T = nc.dram_tensor("pw2T", [128, 8, BN], F16, kind="ExternalInput")
    pb2_row = nc.dram_tensor("pb2_row", [1, BN], F16, kind="ExternalInput")
    ones16 = nc.dram_tensor("ones16", [1, 128], F16, kind="ExternalInput")
    ones32 = nc.dram_tensor("ones32", [1, 128], F32, kind="ExternalInput")
    onescol = nc.dram_tensor("onescol", [128, 1], F32, kind="ExternalInput")
    eye16 = nc.dram_tensor("eye16", [128, 128], F16, kind="ExternalInput")
    eye32 = nc.dram_tensor("eye32", [128, 128], F32, kind="ExternalInput")
    iota8 = nc.dram_tensor("iota8", [128, 8], F32, kind="ExternalInput")

    # ---- outputs ----
    o_idx = nc.dram_tensor("o_idx", [BS, K], I32, kind="ExternalOutput")
    o_qst = nc.dram_tensor("o_qst", [BS, K, C], F32, kind="ExternalOutput")
    o_narr = nc.dram_tensor("o_narr", [BS, BN], F32, kind="ExternalOutput")
    o_unc = nc.dram_tensor("o_unc", [BS, H], F32, kind="ExternalOutput")
    o_pred = nc.dram_tensor("o_pred", [BS, BN], F32, kind="ExternalOutput")
    o_loss = nc.dram_tensor("o_loss", [1, 1], F32, kind="ExternalOutput")
    o_hid = nc.dram_tensor("o_hid", [BS, H], F32, kind="ExternalOutput")

    with tile.TileContext(nc) as tc, ExitStack() as stack:
        cpool = stack.enter_context(tc.tile_pool(name="cpool", bufs=1))
        dyn2 = stack.enter_context(tc.tile_pool(name="dyn2", bufs=2))
        dyng = stack.enter_context(tc.tile_pool(name="dyng", bufs=8))
        dynx = stack.enter_context(tc.tile_pool(name="dynx", bufs=3))

        # consts
        on16_sb = cpool.tile([1, 128], F16, tag="on16")
        on32_sb = cpool.tile([1, 128], F32, tag="on32")
        onc_sb = cpool.tile([128, 1], F32, tag="onc")
        eye16_sb = cpool.tile([128, 128], F16, tag="eye16")
        eye32_sb = cpool.tile([128, 128], F32, tag="eye32")
        iota_sb = cpool.tile([128, 8], F32, tag="iota")
        btr_sb = cpool.tile([128, 512], F32, tag="btr")
        btz_sb = cpool.tile([128, 512], F32, tag="btz")
        btnh_sb = cpool.tile([128, 512], F32, tag="btnh")
        btni_sb = cpool.tile([128, 512], F32, tag="btni")
        zrow16 = cpool.tile([1, 512], F16, tag="zrow16")
        nc.vector.memset(zrow16[:], 0.0)
        zrow32 = cpool.tile([1, 512], F32, tag="zrow32")
        nc.vector.memset(zrow32[:], 0.0)
        for dst, src in ((on16_sb, ones16), (on32_sb, ones32), (onc_sb, onescol),
                         (eye16_sb, eye16), (eye32_sb, eye32), (iota_sb, iota8),
                         (btr_sb, bt_r), (btz_sb, bt_z), (btnh_sb, bt_nh),
                         (btni_sb, bt_ni)):
            nc.sync.dma_start(dst[:], src[:])

        # ---------------- recurrence ----------------
        h2_prev = dyn2.tile([128, 512], F32, tag="h2")
        nc.vector.memset(h2_prev[:], 0.0)

        xt_tiles = {}

        def prefetch_xt(t):
            if t > W:
                return
            xh = dynx.tile([128, 4, BS], F16, tag="xth")
            nc.sync.dma_start(xh[:], xT_hi[:, :, (t - 1) * BS:t * BS])
            xl = dynx.tile([128, 4, BS], F16, tag="xtl")
            nc.sync.dma_start(xl[:], xT_lo[:, :, (t - 1) * BS:t * BS])
            xt_tiles[t] = (xh, xl)

        with tc.tile_pool(name="rw", bufs=1) as rw:
            wih_hi = rw.tile([128, 4, G3], F16, tag="wihhi")
            wih_lo = rw.tile([128, 4, G3], F16, tag="wihlo")
            whh_hi = rw.tile([128, 8, G3], F16, tag="whhhi")
            whh_lo = rw.tile([128, 8, G3], F16, tag="whhlo")
            nc.sync.dma_start(wih_hi[:], wihT_hi[:])
            nc.sync.dma_start(wih_lo[:], wihT_lo[:])
            nc.sync.dma_start(whh_hi[:], whhT_hi[:])
            nc.sync.dma_start(whh_lo[:], whhT_lo[:])

            with tc.tile_pool(name="ghp", bufs=2, space="PSUM") as ghp, \
                 tc.tile_pool(name="smp", bufs=2, space="PSUM") as smp:

                def a_block(t, first_step):
                    """x-projection matmuls (3-term hi/lo) for step t."""
                    xh, xl = xt_tiles.pop(t)
                    gh = ghp.tile([128, 1536], F32, tag="gh")
                    i_n = smp.tile([128, 512], F32, tag="sm")
                    o128 = on16_sb[0:1, 0:128]
                    # one start=True zeroing matmul per bank region
                    nc.tensor.matmul(gh[:, 0:512], o128, zrow16[0:1, :],
                                     start=True, stop=False, skip_group_check=True)
                    nc.tensor.matmul(gh[:, 512:1024], o128, zrow16[0:1, :],
                                     start=True, stop=False, skip_group_check=True)
                    nc.tensor.matmul(gh[:, 1024:1536], o128, zrow16[0:1, :],
                                     start=True, stop=first_step, skip_group_check=True)
                    nc.tensor.matmul(i_n[:, :], o128, zrow16[0:1, :],
                                     start=True, stop=False, skip_group_check=True)
                    terms = ((xh, wih_hi), (xh, wih_lo), (xl, wih_hi))
                    for ti, (xa, wa) in enumerate(terms):
                        for c in range(4):
                            last = (ti == 2 and c == 3)
                            for g0, dst, lo_f, stop in (
                                    (0, gh, 0, last and first_step),
                                    (1024, gh, 512, last and first_step),
                                    (2048, i_n, 0, last)):
                                for col in range(2):
                                    po = col * 64
                                    cs = col * 512
                                    nc.tensor.matmul(
                                        dst[po:po + 64, lo_f:lo_f + 512],
                                        xa[:, c, :],
                                        wa[:, c, g0 + cs:g0 + cs + 512],
                                        start=False, stop=stop,
                                        skip_group_check=True,
                                        tile_position=(0, po))
                    return gh, i_n

                def b_block(gh, hT_prev):
                    """h-recurrence matmuls (3-term hi/lo). Region order: r, hn, z."""
                    def hchunk(c, lo):
                        off = 512 * lo + _hmap(c)
                        return hT_prev[:, off:off + 64]
                    for g0, fr in ((0, 0), (2048, 1024), (1024, 512)):
                        for ti in range(3):
                            wa = whh_lo if ti == 1 else whh_hi
                            hlo = 1 if ti == 2 else 0
                            for c in range(8):
                                hc = hchunk(c, hlo)
                                for col in range(2):
                                    po = col * 64
                                    nc.tensor.matmul(
                                        gh[po:po + 64, fr:fr + 512], hc,
                                        wa[:, c, g0 + col * 512:g0 + col * 512 + 512],
                                        start=False,
                                        stop=(ti == 2 and c == 7),
                                        skip_group_check=True,
                                        tile_position=(0, po))

                def c_block(gh, i_n, h2_in, t):
                    last = (t == W)
                    r_pre = dyng.tile([128, 512], F32, tag="gate")
                    nc.vector.tensor_tensor(r_pre[:], gh[:, 0:512], btr_sb[:], OP.add)
                    r_sb = dyng.tile([128, 512], F32, tag="gate")
                    nc.scalar.activation(r_sb[:], r_pre[:], AF.Sigmoid)
                    t2a = dyng.tile([128, 512], F32, tag="gate")
                    nc.vector.tensor_tensor(t2a[:], i_n[:], btni_sb[:], OP.add)
                    t1 = dyng.tile([128, 512], F32, tag="gate")
                    if t == 1:
                        nc.vector.tensor_tensor(t1[:], btnh_sb[:], r_sb[:], OP.mult)
                    else:
                        t0 = dyng.tile([128, 512], F32, tag="gate")
                        nc.vector.tensor_tensor(t0[:], gh[:, 1024:1536], btnh_sb[:], OP.add)
                        nc.vector.tensor_tensor(t1[:], t0[:], r_sb[:], OP.mult)
                    t2 = dyng.tile([128, 512], F32, tag="gate")
                    nc.vector.tensor_tensor(t2[:], t1[:], t2a[:], OP.add)
                    n_sb = dyng.tile([128, 512], F32, tag="gate")
                    nc.scalar.activation(n_sb[:], t2[:], AF.Tanh)
                    z_pre = dyng.tile([128, 512], F32, tag="gate")
                    nc.vector.tensor_tensor(z_pre[:], gh[:, 512:1024], btz_sb[:], OP.add)
                    z_sb = dyng.tile([128, 512], F32, tag="gate")
                    nc.scalar.activation(z_sb[:], z_pre[:], AF.Sigmoid)
                    v_sb = dyng.tile([128, 512], F32, tag="gate")
                    nc.scalar.activation(v_sb[:], z_pre[:], AF.Sigmoid, scale=-1.0)
                    u_sb = dyng.tile([128, 512], F32, tag="gate")
                    nc.vector.tensor_tensor(u_sb[:], z_sb[:], h2_in[:], OP.mult)
                    w_sb = dyng.tile([128, 512], F32, tag="gate")
                    nc.vector.tensor_tensor(w_sb[:], v_sb[:], n_sb[:], OP.mult)
                    h2_t = dyn2.tile([128, 512], F32, tag="h2")
                    nc.vector.tensor_tensor(h2_t[:], u_sb[:], w_sb[:], OP.add)
                    if last:
                        return h2_t, None, None
                    h2b_hi = dyn2.tile([128, 512], F16, tag="h2bh")
                    nc.scalar.copy(h2b_hi[:], h2_t[:])
                    h2b_lo = dyn2.tile([128, 512], F16, tag="h2bl")
                    nc.vector.tensor_tensor(h2b_lo[:], h2_t[:], h2b_hi[:], OP.subtract)
                    return h2_t, h2b_hi, h2b_lo

                def d_block(h2b_hi, h2b_lo):
                    tr = smp.tile([128, 1024], F16, tag="sm")
                    for j in range(4):
                        nc.tensor.transpose(tr[:, j * 128:(j + 1) * 128],
                                            h2b_hi[:, j * 128:(j + 1) * 128], eye16_sb[:])
                    for j in range(4):
                        nc.tensor.transpose(tr[:, 512 + j * 128:512 + (j + 1) * 128],
                                            h2b_lo[:, j * 128:(j + 1) * 128], eye16_sb[:])
                    hT_t = dyn2.tile([128, 1024], F16, tag="hT")
                    nc.vector.tensor_copy(hT_t[:], tr[:])
                    return hT_t

                prefetch_xt(1)
                prefetch_xt(2)
                gh_t, in_t = a_block(1, True)
                hT_prev = None
                for t in range(1, W + 1):
                    prefetch_xt(t + 2)
                    if t > 1:
                        b_block(gh_t, hT_prev)
                    h2_t, h2b_hi, h2b_lo = c_block(gh_t, in_t, h2_prev, t)
                    if t < W:
                        gh_t, in_t = a_block(t + 1, False)
                        hT_prev = d_block(h2b_hi, h2b_lo)
                    h2_prev = h2_t

        h2f = h2_prev  # [128, 512] f32: p<64 -> h[b, 0:512]; p>=64 -> h[b, 512:1024]
        nc.sync.dma_start(o_hid[:, 0:512], h2f[0:64, :])
        nc.sync.dma_start(o_hid[:, 512:1024], h2f[64:128, :])

        # tail weights (allocated after the recurrence weights free their space)
        tailp = stack.enter_context(tc.tile_pool(name="tailp", bufs=1))
        wq_sb = tailp.tile([128, 8, BN], F32, tag="wq")
        uw1_sb = tailp.tile([128, 8, H], F16, tag="uw1")
        embT2_sb = tailp.tile([128, NCODE], F32, tag="embT2")
        nn_sb = tailp.tile([1, NCODE], F32, tag="negnorm")
        emb8_sb = tailp.tile([128, 8, C], F32, tag="emb8")
        lw_sb = tailp.tile([128, 4, BS], F16, tag="lw")
        uw2_sb = tailp.tile([128, 8, H], F16, tag="uw2")
        pw1_sb = tailp.tile([128, 4, H], F16, tag="pw1")
        pw2_sb = tailp.tile([128, 8, BN], F16, tag="pw2")
        bq_sb = tailp.tile([1, BN], F32, tag="bq")
        ub1_sb = tailp.tile([1, H], F16, tag="ub1")
        ub2_sb = tailp.tile([1, H], F16, tag="ub2")
        pb1_sb = tailp.tile([1, H], F16, tag="pb1")
        pb2_sb = tailp.tile([1, BN], F16, tag="pb2")
        for dst, src in ((wq_sb, wqT), (uw1_sb, uw1T), (embT2_sb, embT2),
                         (nn_sb, negnorm), (emb8_sb, emb8), (lw_sb, lwT16),
                         (uw2_sb, uw2T), (pw1_sb, pw1T), (pw2_sb, pw2T),
                         (bq_sb, bq_row), (ub1_sb, ub1_row), (ub2_sb, ub2_row),
                         (pb1_sb, pb1_row), (pb2_sb, pb2_row)):
            nc.sync.dma_start(dst[:], src[:])

        # ---------------- tail ----------------
        with tc.tile_pool(name="pstail", bufs=1, space="PSUM") as pst, \
             tc.tile_pool(name="pstr1", bufs=1, space="PSUM") as pstr1, \
             tc.tile_pool(name="pstr2", bufs=2, space="PSUM") as pstr2, \
             tc.tile_pool(name="dscr", bufs=1, space="DRAM") as dscr:

            # fp32 transposed final hidden
            trh = pstr1.tile([128, 512], F32, tag="tr")
            for j in range(4):
                nc.tensor.transpose(trh[:, j * 128:(j + 1) * 128],
                                    h2f[:, j * 128:(j + 1) * 128], eye32_sb[:])
            hT32 = tailp.tile([128, 512], F32, tag="hT32")
            nc.vector.tensor_copy(hT32[:], trh[:])

            # queries q = h @ Wq.T + bq  [64, 512]
            q_ps = pst.tile([128, BN], F32, tag="pq")
            nc.tensor.matmul(q_ps[0:64, :], on32_sb[0:1, 0:64], bq_sb[0:1, :],
                             start=True, stop=False)
            for c in range(8):
                nc.tensor.matmul(q_ps[0:64, :], hT32[:, _hmap(c):_hmap(c) + 64],
                                 wq_sb[:, c, :], start=False, stop=(c == 7))
            q_sb = tailp.tile([128, BN], F32, tag="q")
            nc.vector.tensor_copy(q_sb[0:64, :], q_ps[0:64, :])

            # qT chunks: [128, 4, 64], chunk j partitions = (k=2j: c | k=2j+1: c)
            trq = pstr1.tile([128, 4, C], F32, tag="trq")
            for j in range(4):
                nc.tensor.transpose(trq[:, j, :], q_sb[0:64, j * 128:(j + 1) * 128],
                                    eye32_sb[0:64, 0:64])
            qT_sb = tailp.tile([128, 4, C], F32, tag="qT")
            nc.vector.tensor_copy(qT_sb[:], trq[:])

            # scores S_j = 2 q.e - |e|^2 - 1e-9 n  [128 (2 k), 1024 codes]
            idx_bounce = dscr.tile([1, 512], F32, tag="idxb")
            for j in range(4):
                S_ps = pst.tile([128, NCODE], F32, tag="pt")
                for ns in range(2):
                    nc.tensor.matmul(
                        S_ps[:, ns * 512:(ns + 1) * 512],
                        on32_sb[0:1, 0:128],
                        nn_sb[0:1, ns * 512:(ns + 1) * 512],
                        start=True, stop=False, skip_group_check=True)
                    for col in range(2):
                        po = col * 64
                        nc.tensor.matmul(
                            S_ps[po:po + 64, ns * 512:(ns + 1) * 512],
                            qT_sb[po:po + 64, j, :],
                            embT2_sb[po:po + 64, ns * 512:(ns + 1) * 512],
                            start=False, stop=True, skip_group_check=True,
                            tile_position=(po, po))
                S_sb = tailp.tile([128, NCODE], F32, tag="S")
                nc.vector.tensor_copy(S_sb[:], S_ps[:])
                mx = tailp.tile([128, 8], F32, tag="mx")
                mi = tailp.tile([128, 8], U32, tag="mi")
                nc.vector.max_with_indices(mx[:], mi[:], S_sb[:])
                nc.sync.dma_start(o_idx[:, 2 * j:2 * j + 1], mi[0:64, 0:1].bitcast(I32))
                nc.sync.dma_start(o_idx[:, 2 * j + 1:2 * j + 2], mi[64:128, 0:1].bitcast(I32))
                idxf = tailp.tile([128, 1], F32, tag="idxf")
                nc.vector.tensor_copy(idxf[:], mi[:, 0:1])
                nc.sync.dma_start(idx_bounce[0:1, j * 128:(j + 1) * 128], idxf[:, 0:1])

            idxrow = tailp.tile([1, 512], F32, tag="idxrow")
            nc.sync.dma_start(idxrow[:], idx_bounce[:])

            # broadcast idx over partitions; q index order along free = (k, b)
            bc_ps = pst.tile([128, 512], F32, tag="pq")
            nc.tensor.matmul(bc_ps[:], on32_sb[0:1, :], idxrow[0:1, :],
                             start=True, stop=True)
            bc_sb = dyng.tile([128, 512], F32, tag="gate")
            nc.vector.tensor_copy(bc_sb[:], bc_ps[:])

            # one-hot matmul -> narr [64, (k c)] (exact fp32 emb rows)
            narr_ps = pst.tile([128, BN], F32, tag="pq")
            nc.tensor.matmul(narr_ps[0:64, :], on32_sb[0:1, 0:64], zrow32[0:1, :],
                             start=True, stop=False)
            for nci in range(8):
                oh = dyng.tile([128, 512], F32, tag="gate")
                nc.vector.tensor_scalar(oh[:], bc_sb[:], iota_sb[:, nci:nci + 1], None,
                                        op0=OP.is_equal)
                for k in range(8):
                    nc.tensor.matmul(narr_ps[0:64, k * C:(k + 1) * C],
                                     oh[:, k * 64:(k + 1) * 64], emb8_sb[:, nci, :],
                                     start=False, stop=(nci == 7 and k == 7))
            narr_sb = tailp.tile([128, BN], F32, tag="narr")
            nc.vector.tensor_copy(narr_sb[0:64, :], narr_ps[0:64, :])
            nc.sync.dma_start(o_qst.rearrange("b k c -> b (k c)"), narr_sb[0:64, :])
            nc.sync.dma_start(o_narr[:], narr_sb[0:64, :])

            # vq loss partial: sum((q - narr)^2)
            d_sb = dyng.tile([128, BN], F32, tag="gate")
            nc.vector.tensor_tensor(d_sb[0:64, :], q_sb[0:64, :], narr_sb[0:64, :],
                                    OP.subtract)
            d2 = dyng.tile([128, BN], F32, tag="gate")
            dcol = tailp.tile([128, 1], F32, tag="dcol")
            nc.vector.scalar_tensor_tensor(d2[0:64, :], d_sb[0:64, :], 0.0,
                                           d_sb[0:64, :], op0=OP.add, op1=OP.mult,
                                           accum_out=dcol[0:64, :])
            loss_ps = pst.tile([128, 512], F32, tag="pq")
            nc.tensor.matmul(loss_ps[0:1, 0:1], dcol[0:64, 0:1], onc_sb[0:64, 0:1],
                             start=True, stop=True)
            loss_sb = tailp.tile([1, 1], F32, tag="loss")
            nc.vector.tensor_copy(loss_sb[:], loss_ps[0:1, 0:1])
            nc.sync.dma_start(o_loss[:], loss_sb[:])

            # narrT fp16 chunks for the heads
            narr16 = tailp.tile([128, BN], F16, tag="narr16")
            nc.scalar.copy(narr16[0:64, :], narr_sb[0:64, :])
            trn = pstr2.tile([128, 4, C], F16, tag="tr16")
            for j in range(4):
                nc.tensor.transpose(trn[:, j, :], narr16[0:64, j * 128:(j + 1) * 128],
                                    eye16_sb[0:64, 0:64])
            narrT = tailp.tile([128, 4, C], F16, tag="narrT")
            nc.vector.tensor_copy(narrT[:], trn[:])

            def head_layer(out_ps, in_chunks, w_sb_, b_sb_, nslices, nk):
                for ns in range(nslices):
                    nc.tensor.matmul(out_ps[0:64, ns * 512:(ns + 1) * 512],
                                     on16_sb[0:1, 0:64],
                                     b_sb_[0:1, ns * 512:(ns + 1) * 512],
                                     start=True, stop=False)
                    for c in range(nk):
                        nc.tensor.matmul(out_ps[0:64, ns * 512:(ns + 1) * 512],
                                         in_chunks(c),
                                         w_sb_[:, c, ns * 512:(ns + 1) * 512],
                                         start=False, stop=(c == nk - 1))

            def transpose8(src_sb, tag):
                """[64, 1024] f16 -> [128, 8, 64] f16 chunks."""
                tr_a = pstr2.tile([128, 4, C], F16, tag="tr16")
                for j in range(4):
                    nc.tensor.transpose(tr_a[:, j, :],
                                        src_sb[0:64, j * 128:(j + 1) * 128],
                                        eye16_sb[0:64, 0:64])
                tr_b = pstr2.tile([128, 4, C], F16, tag="tr16")
                for j in range(4):
                    nc.tensor.transpose(tr_b[:, j, :],
                                        src_sb[0:64, 512 + j * 128:512 + (j + 1) * 128],
                                        eye16_sb[0:64, 0:64])
                dst = tailp.tile([128, 8, C], F16, tag=tag)
                nc.vector.tensor_copy(dst[:, 0:4, :], tr_a[:])
                nc.vector.tensor_copy(dst[:, 4:8, :], tr_b[:])
                return dst

            # uncertainty head: silu(u_in @ uW1.T + ub1) @ uW2.T + ub2
            u1_ps = pst.tile([128, H], F32, tag="pt")
            head_layer(u1_ps,
                       lambda c: lw_sb[:, c, :] if c < 4 else narrT[:, c - 4, :],
                       uw1_sb, ub1_sb, 2, 8)
            s1 = tailp.tile([128, H], F16, tag="s1")
            nc.scalar.activation(s1[0:64, :], u1_ps[0:64, :], AF.Silu)
            s1T = transpose8(s1, "s1T")
            u2_ps = pst.tile([128, H], F32, tag="pt")
            head_layer(u2_ps, lambda c: s1T[:, c, :], uw2_sb, ub2_sb, 2, 8)
            u2_sb = tailp.tile([128, H], F32, tag="u2")
            nc.vector.tensor_copy(u2_sb[0:64, :], u2_ps[0:64, :])
            nc.sync.dma_start(o_unc[:], u2_sb[0:64, :])

            # prediction head: silu(narr @ pW1.T + pb1) @ pW2.T + pb2
            p1_ps = pst.tile([128, H], F32, tag="pt")
            head_layer(p1_ps, lambda c: narrT[:, c, :], pw1_sb, pb1_sb, 2, 4)
            s2 = tailp.tile([128, H], F16, tag="s2")
            nc.scalar.activation(s2[0:64, :], p1_ps[0:64, :], AF.Silu)
            s2T = transpose8(s2, "s2T")
            p2_ps = pst.tile([128, BN], F32, tag="pq")
            head_layer(p2_ps, lambda c: s2T[:, c, :], pw2_sb, pb2_sb, 1, 8)
            p2_sb = tailp.tile([128, BN], F32, tag="p2")
            nc.vector.tensor_copy(p2_sb[0:64, :], p2_ps[0:64, :])
            nc.sync.dma_start(o_pred[:], p2_sb[0:64, :])

    nc.compile()
    return nc


def _chunk3(M):
    """[n*128, X] -> [128, n, X] contiguous."""
    n = M.shape[0] // 128
    return np.ascontiguousarray(M.reshape(n, 128, -1).transpose(1, 0, 2))


def _prep_shared(inputs):
    f16 = np.float16
    f32 = np.float32
    W_ih = np.asarray(inputs["W_ih"], f32)
    W_hh = np.asarray(inputs["W_hh"], f32)
    b_ih = np.asarray(inputs["b_ih"], f32)
    b_hh = np.asarray(inputs["b_hh"], f32)
    Wq = np.asarray(inputs["Wq"], f32)
    bq = np.asarray(inputs["bq"], f32)
    emb = np.asarray(inputs["emb"], f32)
    uW1 = np.asarray(inputs["uW1"], f32)
    uW2 = np.asarray(inputs["uW2"], f32)
    pW1 = np.asarray(inputs["pW1"], f32)
    pW2 = np.asarray(inputs["pW2"], f32)
    p = np.arange(128, dtype=f32)

    def bt(vec):
        # [1024] bias -> [128, 512] broadcast tile (col-split layout)
        out = np.empty((128, 512), f32)
        out[0:64, :] = vec[None, 0:512]
        out[64:128, :] = vec[None, 512:1024]
        return out

    brz = b_ih + b_hh
    wihT = W_ih.T  # [512, 3072]
    whhT = W_hh.T  # [1024, 3072]
    wih_hi = wihT.astype(f16)
    wih_lo = (wihT - wih_hi.astype(f32)).astype(f16)
    whh_hi = whhT.astype(f16)
    whh_lo = (whhT - whh_hi.astype(f32)).astype(f16)
    return {
        "wihT_hi": _chunk3(wih_hi),
        "wihT_lo": _chunk3(wih_lo),
        "whhT_hi": _chunk3(whh_hi),
        "whhT_lo": _chunk3(whh_lo),
        "bt_r": bt(brz[0:1024]),
        "bt_z": bt(brz[1024:2048]),
        "bt_nh": bt(b_hh[2048:3072]),
        "bt_ni": bt(b_ih[2048:3072]),
        "wqT": _chunk3(Wq.T).astype(f32),
        "bq_row": bq[None, :].astype(f32),
        "embT2": np.concatenate([2.0 * emb.T, 2.0 * emb.T], axis=0).astype(f32),
        "negnorm": (-(emb.astype(np.float64) ** 2).sum(1)
                    - 1e-9 * np.arange(NCODE))[None, :].astype(f32),
        "emb8": _chunk3(emb).astype(f32),
        "uw1T": _chunk3(uW1.T).astype(f16),
        "ub1_row": np.asarray(inputs["ub1"], f32)[None, :].astype(f16),
        "uw2T": _chunk3(uW2.T).astype(f16),
        "ub2_row": np.asarray(inputs["ub2"], f32)[None, :].astype(f16),
        "pw1T": _chunk3(pW1.T).astype(f16),
        "pb1_row": np.asarray(inputs["pb1"], f32)[None, :].astype(f16),
        "pw2T": _chunk3(pW2.T).astype(f16),
        "pb2_row": np.asarray(inputs["pb2"], f32)[None, :].astype(f16),
        "ones16": np.ones((1, 128), f16),
        "ones32": np.ones((1, 128), f32),
        "onescol": np.ones((128, 1), f32),
        "eye16": np.eye(128, dtype=f16),
        "eye32": np.eye(128, dtype=f32),
        "iota8": (p[:, None] + 128.0 * np.arange(8, dtype=f32)[None, :]).astype(f32),
    }


def kernel(**inputs):
    if "nc" not in _CACHE:
        _CACHE["nc"] = _build()
    nc = _CACHE["nc"]

    x = np.asarray(inputs["state_window"], np.float32)
    shared = _prep_shared(inputs)
    in_maps = []
    for ci in range(NCORES):
        shard = x[ci * BS:(ci + 1) * BS]          # [64, 64, 512]
        xt = shard.transpose(2, 1, 0)             # [512 d, 64 w, 64 b]
        m = dict(shared)
        xflat = np.ascontiguousarray(xt.reshape(D, W * BS))
        x_hi = xflat.astype(np.float16)
        x_lo = (xflat - x_hi.astype(np.float32)).astype(np.float16)
        m["xT_hi"] = _chunk3(x_hi)
        m["xT_lo"] = _chunk3(x_lo)
        m["lwT16"] = _chunk3(np.ascontiguousarray(shard[:, -1, :].T)).astype(np.float16)
        in_maps.append(m)

    res = run_bass_kernel_spmd(nc, in_maps, list(range(NCORES)))
    kernel.LAST_RESULT = res

    r = res.results
    code_indices = np.concatenate([r[c]["o_idx"] for c in range(NCORES)], axis=0)
    quantized_st = np.concatenate([r[c]["o_qst"] for c in range(NCORES)], axis=0)
    narrator = np.concatenate([r[c]["o_narr"] for c in range(NCORES)], axis=0)
    uncertainty = np.concatenate([r[c]["o_unc"] for c in range(NCORES)], axis=0)
    predicted = np.concatenate([r[c]["o_pred"] for c in range(NCORES)], axis=0)
    last_hidden = np.concatenate([r[c]["o_hid"] for c in range(NCORES)], axis=0)
    total = sum(float(r[c]["o_loss"][0, 0]) for c in range(NCORES))
    vq_loss = np.float32(1.25 * total / (B * K * C))
    return (code_indices.astype(np.int32), quantized_st, narrator, uncertainty,
            predicted, vq_loss, last_hidden)


# revision 21
# speedup vs baseline: 1.3753x; 1.0067x over previous
"""Trainium2 Bass kernel for nn_DiscreteNarrator (GRU + VQ codebook + heads).

Strategy: data-parallel over batch across 8 NeuronCores (64 rows/core).
Per core:
  - GRU recurrence runs fully on-chip. Layout: gh[b, g] with batch on
    PSUM partitions, gate slices on the free dim; dual column-group
    matmuls (tile_position) pack two independent 512-wide gate slices
    into one 128x128 PE pass so the 64-row batch uses the full array.
  - Matmul inputs in fp16 (10-bit mantissa; verified 0 argmin flips vs
    fp32 reference on the generated codebook margins), fp32 PSUM
    accumulation. The x-projection for step t+1 is fused into step t's
    PE stream (no separate gi_all phase, no DRAM staging).
  - VQ tail in fp32: queries, scores S = 2 q.e - |e|^2 - 1e-9*n (argmax
    == argmin-with-first-index-tie-break), one-hot matmul against the
    fp32 codebook so quantized rows are exact; heads use fp16 weights.
vq_loss partial sums are reduced on the host (the only cross-core term).
"""
import os
import sys
import types

import numpy as np

_HERE = os.path.dirname(os.path.abspath(__file__))


def _register_ntff_hook():
    """Make trace=True (BASS_TRACE=1) work under axon if the hook is absent."""
    try:
        from antenv.axon_hooks import get_axon_ntff_profile_hook  # noqa
        return
    except Exception:
        pass
    try:
        from trn_agent_boot.trn_boot import _ntff_profile_via_ctypes
        hook = _ntff_profile_via_ctypes("/opt/axon/libaxon_pjrt.so")
        mod = types.ModuleType("antenv.axon_hooks")
        mod.get_axon_ntff_profile_hook = lambda: hook
        mod.set_axon_ntff_profile_hook = lambda h: None
        import antenv
        sys.modules["antenv.axon_hooks"] = mod
        antenv.axon_hooks = mod
    except Exception:
        pass


_register_ntff_hook()

import concourse.tile as tile
import concourse.mybir as mybir
from concourse import bacc
from concourse.bass_utils import run_bass_kernel_spmd

F32 = mybir.dt.float32
F16 = mybir.dt.float16
U32 = mybir.dt.uint32
I32 = mybir.dt.int32
AF = mybir.ActivationFunctionType
OP = mybir.AluOpType

# problem dims (hardcoded per contract)
B, W, D, H = 512, 64, 512, 1024
NCODE, K, C = 1024, 8, 64
BN = K * C          # 512
G3 = 3 * H          # 3072
NCORES = 8
BS = B // NCORES    # 64

_CACHE = {}


def _hmap(c):
    """Free-dim offset of hT chunk c inside the [128, 512] hT tile."""
    return (c % 4) * 128 + (c // 4) * 64


def _build():
    from contextlib import ExitStack

    nc = bacc.Bacc(None, target_bir_lowering=False)

    # ---- inputs ----
    xT_hi = nc.dram_tensor("xT_hi", [128, 4, W * BS], F16, kind="ExternalInput")
    xT_lo = nc.dram_tensor("xT_lo", [128, 4, W * BS], F16, kind="ExternalInput")
    wihT_hi = nc.dram_tensor("wihT_hi", [128, 4, G3], F16, kind="ExternalInput")
    wihT_lo = nc.dram_tensor("wihT_lo", [128, 4, G3], F16, kind="ExternalInput")
    whhT_hi = nc.dram_tensor("whhT_hi", [128, 8, G3], F16, kind="ExternalInput")
    whhT_lo = nc.dram_tensor("whhT_lo", [128, 8, G3], F16, kind="ExternalInput")
    bt_r = nc.dram_tensor("bt_r", [128, 512], F32, kind="ExternalInput")
    bt_z = nc.dram_tensor("bt_z", [128, 512], F32, kind="ExternalInput")
    bt_nh = nc.dram_tensor("bt_nh", [128, 512], F32, kind="ExternalInput")
    bt_ni = nc.dram_tensor("bt_ni", [128, 512], F32, kind="ExternalInput")
    wqT = nc.dram_tensor("wqT", [128, 8, BN], F32, kind="ExternalInput")
    bq_row = nc.dram_tensor("bq_row", [1, BN], F32, kind="ExternalInput")
    embT2 = nc.dram_tensor("embT2", [128, NCODE], F32, kind="ExternalInput")
    negnorm = nc.dram_tensor("negnorm", [1, NCODE], F32, kind="ExternalInput")
    emb8 = nc.dram_tensor("emb8", [128, 8, C], F32, kind="ExternalInput")
    lwT16 = nc.dram_tensor("lwT16", [128, 4, BS], F16, kind="ExternalInput")
    uw1T = nc.dram_tensor("uw1T", [128, 8, H], F16, kind="ExternalInput")
    ub1_row = nc.dram_tensor("ub1_row", [1, H], F16, kind="ExternalInput")
    uw2T = nc.dram_tensor("uw2T", [128, 8, H], F16, kind="ExternalInput")
    ub2_row = nc.dram_tensor("ub2_row", [1, H], F16, kind="ExternalInput")
    pw1T = nc.dram_tensor("pw1T", [128, 4, H], F16, kind="ExternalInput")
    pb1_row = nc.dram_tensor("pb1_row", [1, H], F16, kind="ExternalInput")
    pw2T = nc.dram_tensor("pw2T", [128, 8, BN], F16, kind="ExternalInput")
    pb2_row = nc.dram_tensor("pb2_row", [1, BN], F16, kind="ExternalInput")
    ones16 = nc.dram_tensor("ones16", [1, 128], F16, kind="ExternalInput")
    ones32 = nc.dram_tensor("ones32", [1, 128], F32, kind="ExternalInput")
    onescol = nc.dram_tensor("onescol", [128, 1], F32, kind="ExternalInput")
    eye16 = nc.dram_tensor("eye16", [128, 128], F16, kind="ExternalInput")
    eye32 = nc.dram_tensor("eye32", [128, 128], F32, kind="ExternalInput")
    iota8 = nc.dram_tensor("iota8", [128, 8], F32, kind="ExternalInput")

    # ---- outputs ----
    o_idx = nc.dram_tensor("o_idx", [BS, K], I32, kind="ExternalOutput")
    o_qst = nc.dram_tensor("o_qst", [BS, K, C], F32, kind="ExternalOutput")
    o_narr = nc.dram_tensor("o_narr", [BS, BN], F32, kind="ExternalOutput")
    o_unc = nc.dram_tensor("o_unc", [BS, H], F32, kind="ExternalOutput")
    o_pred = nc.dram_tensor("o_pred", [BS, BN], F32, kind="ExternalOutput")
    o_loss = nc.dram_tensor("o_loss", [1, 1], F32, kind="ExternalOutput")
    o_hid = nc.dram_tensor("o_hid", [BS, H], F32, kind="ExternalOutput")

    with tile.TileContext(nc) as tc, ExitStack() as stack:
        cpool = stack.enter_context(tc.tile_pool(name="cpool", bufs=1))
        dyn2 = stack.enter_context(tc.tile_pool(name="dyn2", bufs=2))
        dyng = stack.enter_context(tc.tile_pool(name="dyng", bufs=8))
        dynx = stack.enter_context(tc.tile_pool(name="dynx", bufs=3))

        # consts
        on16_sb = cpool.tile([1, 128], F16, tag="on16")
        on32_sb = cpool.tile([1, 128], F32, tag="on32")
        onc_sb = cpool.tile([128, 1], F32, tag="onc")
        eye16_sb = cpool.tile([128, 128], F16, tag="eye16")
        eye32_sb = cpool.tile([128, 128], F32, tag="eye32")
        iota_sb = cpool.tile([128, 8], F32, tag="iota")
        btr_sb = cpool.tile([128, 512], F32, tag="btr")
        btz_sb = cpool.tile([128, 512], F32, tag="btz")
        btnh_sb = cpool.tile([128, 512], F32, tag="btnh")
        btni_sb = cpool.tile([128, 512], F32, tag="btni")
        zrow16 = cpool.tile([1, 512], F16, tag="zrow16")
        nc.vector.memset(zrow16[:], 0.0)
        zrow32 = cpool.tile([1, 512], F32, tag="zrow32")
        nc.vector.memset(zrow32[:], 0.0)
        for dst, src in ((on16_sb, ones16), (on32_sb, ones32), (onc_sb, onescol),
                         (eye16_sb, eye16), (eye32_sb, eye32), (iota_sb, iota8),
                         (btr_sb, bt_r), (btz_sb, bt_z), (btnh_sb, bt_nh),
                         (btni_sb, bt_ni)):
            nc.sync.dma_start(dst[:], src[:])

        # ---------------- recurrence ----------------
        h2_prev = dyn2.tile([128, 512], F32, tag="h2")
        nc.vector.memset(h2_prev[:], 0.0)

        xt_tiles = {}

        def prefetch_xt(t):
            if t > W:
                return
            xh = dynx.tile([128, 4, BS], F16, tag="xth")
            nc.gpsimd.dma_start(xh[:], xT_hi[:, :, (t - 1) * BS:t * BS])
            xl = dynx.tile([128, 4, BS], F16, tag="xtl")
            nc.gpsimd.dma_start(xl[:], xT_lo[:, :, (t - 1) * BS:t * BS])
            xt_tiles[t] = (xh, xl)

        with tc.tile_pool(name="rw", bufs=1) as rw:
            wih_hi = rw.tile([128, 4, G3], F16, tag="wihhi")
            wih_lo = rw.tile([128, 4, G3], F16, tag="wihlo")
            whh_hi = rw.tile([128, 8, G3], F16, tag="whhhi")
            whh_lo = rw.tile([128, 8, G3], F16, tag="whhlo")
            nc.sync.dma_start(wih_hi[:], wihT_hi[:])
            nc.sync.dma_start(wih_lo[:], wihT_lo[:])
            nc.sync.dma_start(whh_hi[:], whhT_hi[:])
            nc.sync.dma_start(whh_lo[:], whhT_lo[:])

            with tc.tile_pool(name="ghp", bufs=2, space="PSUM") as ghp, \
                 tc.tile_pool(name="smp", bufs=2, space="PSUM") as smp:

                def a_block(t, first_step):
                    """x-projection matmuls (3-term hi/lo) for step t."""
                    xh, xl = xt_tiles.pop(t)
                    gh = ghp.tile([128, 1536], F32, tag="gh")
                    i_n = smp.tile([128, 512], F32, tag="sm")
                    o128 = on16_sb[0:1, 0:128]
                    # one start=True zeroing matmul per bank region
                    nc.tensor.matmul(gh[:, 0:512], o128, zrow16[0:1, :],
                                     start=True, stop=False, skip_group_check=True)
                    nc.tensor.matmul(gh[:, 512:1024], o128, zrow16[0:1, :],
                                     start=True, stop=False, skip_group_check=True)
                    nc.tensor.matmul(gh[:, 1024:1536], o128, zrow16[0:1, :],
                                     start=True, stop=first_step, skip_group_check=True)
                    nc.tensor.matmul(i_n[:, :], o128, zrow16[0:1, :],
                                     start=True, stop=False, skip_group_check=True)
                    terms = ((xh, wih_hi), (xh, wih_lo), (xl, wih_hi))
                    for ti, (xa, wa) in enumerate(terms):
                        for c in range(4):
                            last = (ti == 2 and c == 3)
                            for g0, dst, lo_f, stop in (
                                    (0, gh, 0, last and first_step),
                                    (1024, gh, 512, last and first_step),
                                    (2048, i_n, 0, last)):
                                for col in range(2):
                                    po = col * 64
                                    cs = col * 512
                                    nc.tensor.matmul(
                                        dst[po:po + 64, lo_f:lo_f + 512],
                                        xa[:, c, :],
                                        wa[:, c, g0 + cs:g0 + cs + 512],
                                        start=False, stop=stop,
                                        skip_group_check=True,
                                        tile_position=(0, po))
                    return gh, i_n

                def b_block(gh, hT_prev):
                    """h-recurrence matmuls (3-term hi/lo). Region order: r, hn, z."""
                    def hchunk(c, lo):
                        off = 512 * lo + _hmap(c)
                        return hT_prev[:, off:off + 64]
                    for g0, fr in ((0, 0), (2048, 1024), (1024, 512)):
                        for ti in range(3):
                            wa = whh_lo if ti == 1 else whh_hi
                            hlo = 1 if ti == 2 else 0
                            for c in range(8):
                                hc = hchunk(c, hlo)
                                for col in range(2):
                                    po = col * 64
                                    nc.tensor.matmul(
                                        gh[po:po + 64, fr:fr + 512], hc,
                                        wa[:, c, g0 + col * 512:g0 + col * 512 + 512],
                                        start=False,
                                        stop=(ti == 2 and c == 7),
                                        skip_group_check=True,
                                        tile_position=(0, po))

                def c_block(gh, i_n, h2_in, t):
                    last = (t == W)
                    r_pre = dyng.tile([128, 512], F32, tag="gate")
                    nc.vector.tensor_tensor(r_pre[:], gh[:, 0:512], btr_sb[:], OP.add)
                    r_sb = dyng.tile([128, 512], F32, tag="gate")
                    nc.scalar.activation(r_sb[:], r_pre[:], AF.Sigmoid)
                    t2a = dyng.tile([128, 512], F32, tag="gate")
                    nc.vector.tensor_tensor(t2a[:], i_n[:], btni_sb[:], OP.add)
                    t1 = dyng.tile([128, 512], F32, tag="gate")
                    if t == 1:
                        nc.vector.tensor_tensor(t1[:], btnh_sb[:], r_sb[:], OP.mult)
                    else:
                        t0 = dyng.tile([128, 512], F32, tag="gate")
                        nc.vector.tensor_tensor(t0[:], gh[:, 1024:1536], btnh_sb[:], OP.add)
                        nc.vector.tensor_tensor(t1[:], t0[:], r_sb[:], OP.mult)
                    t2 = dyng.tile([128, 512], F32, tag="gate")
                    nc.vector.tensor_tensor(t2[:], t1[:], t2a[:], OP.add)
                    n_sb = dyng.tile([128, 512], F32, tag="gate")
                    nc.scalar.activation(n_sb[:], t2[:], AF.Tanh)
                    z_pre = dyng.tile([128, 512], F32, tag="gate")
                    nc.vector.tensor_tensor(z_pre[:], gh[:, 512:1024], btz_sb[:], OP.add)
                    z_sb = dyng.tile([128, 512], F32, tag="gate")
                    nc.scalar.activation(z_sb[:], z_pre[:], AF.Sigmoid)
                    v_sb = dyng.tile([128, 512], F32, tag="gate")
                    nc.scalar.activation(v_sb[:], z_pre[:], AF.Sigmoid, scale=-1.0)
                    u_sb = dyng.tile([128, 512], F32, tag="gate")
                    nc.vector.tensor_tensor(u_sb[:], z_sb[:], h2_in[:], OP.mult)
                    w_sb = dyng.tile([128, 512], F32, tag="gate")
                    nc.vector.tensor_tensor(w_sb[:], v_sb[:], n_sb[:], OP.mult)
                    h2_t = dyn2.tile([128, 512], F32, tag="h2")
                    nc.vector.tensor_tensor(h2_t[:], u_sb[:], w_sb[:], OP.add)
                    if last:
                        return h2_t, None, None
                    h2b_hi = dyn2.tile([128, 512], F16, tag="h2bh")
                    nc.scalar.copy(h2b_hi[:], h2_t[:])
                    h2b_lo = dyn2.tile([128, 512], F16, tag="h2bl")
                    nc.vector.tensor_tensor(h2b_lo[:], h2_t[:], h2b_hi[:], OP.subtract)
                    return h2_t, h2b_hi, h2b_lo

                def d_block(h2b_hi, h2b_lo):
                    tr = smp.tile([128, 1024], F16, tag="sm")
                    for j in range(4):
                        nc.tensor.transpose(tr[:, j * 128:(j + 1) * 128],
                                            h2b_hi[:, j * 128:(j + 1) * 128], eye16_sb[:])
                    for j in range(4):
                        nc.tensor.transpose(tr[:, 512 + j * 128:512 + (j + 1) * 128],
                                            h2b_lo[:, j * 128:(j + 1) * 128], eye16_sb[:])
                    hT_t = dyn2.tile([128, 1024], F16, tag="hT")
                    nc.vector.tensor_copy(hT_t[:], tr[:])
                    return hT_t

                prefetch_xt(1)
                prefetch_xt(2)
                gh_t, in_t = a_block(1, True)
                hT_prev = None
                for t in range(1, W + 1):
                    prefetch_xt(t + 2)
                    if t > 1:
                        b_block(gh_t, hT_prev)
                    h2_t, h2b_hi, h2b_lo = c_block(gh_t, in_t, h2_prev, t)
                    if t < W:
                        gh_t, in_t = a_block(t + 1, False)
                        hT_prev = d_block(h2b_hi, h2b_lo)
                    h2_prev = h2_t

        h2f = h2_prev  # [128, 512] f32: p<64 -> h[b, 0:512]; p>=64 -> h[b, 512:1024]
        nc.sync.dma_start(o_hid[:, 0:512], h2f[0:64, :])
        nc.sync.dma_start(o_hid[:, 512:1024], h2f[64:128, :])

        # tail weights (allocated after the recurrence weights free their space)
        tailp = stack.enter_context(tc.tile_pool(name="tailp", bufs=1))
        wq_sb = tailp.tile([128, 8, BN], F32, tag="wq")
        uw1_sb = tailp.tile([128, 8, H], F16, tag="uw1")
        embT2_sb = tailp.tile([128, NCODE], F32, tag="embT2")
        nn_sb = tailp.tile([1, NCODE], F32, tag="negnorm")
        emb8_sb = tailp.tile([128, 8, C], F32, tag="emb8")
        lw_sb = tailp.tile([128, 4, BS], F16, tag="lw")
        uw2_sb = tailp.tile([128, 8, H], F16, tag="uw2")
        pw1_sb = tailp.tile([128, 4, H], F16, tag="pw1")
        pw2_sb = tailp.tile([128, 8, BN], F16, tag="pw2")
        bq_sb = tailp.tile([1, BN], F32, tag="bq")
        ub1_sb = tailp.tile([1, H], F16, tag="ub1")
        ub2_sb = tailp.tile([1, H], F16, tag="ub2")
        pb1_sb = tailp.tile([1, H], F16, tag="pb1")
        pb2_sb = tailp.tile([1, BN], F16, tag="pb2")
        for dst, src in ((wq_sb, wqT), (uw1_sb, uw1T), (embT2_sb, embT2),
                         (nn_sb, negnorm), (emb8_sb, emb8), (lw_sb, lwT16),
                         (uw2_sb, uw2T), (pw1_sb, pw1T), (pw2_sb, pw2T),
                         (bq_sb, bq_row), (ub1_sb, ub1_row), (ub2_sb, ub2_row),
                         (pb1_sb, pb1_row), (pb2_sb, pb2_row)):
            nc.sync.dma_start(dst[:], src[:])

        # ---------------- tail ----------------
        with tc.tile_pool(name="pstail", bufs=1, space="PSUM") as pst, \
             tc.tile_pool(name="pstr1", bufs=1, space="PSUM") as pstr1, \
             tc.tile_pool(name="pstr2", bufs=2, space="PSUM") as pstr2, \
             tc.tile_pool(name="dscr", bufs=1, space="DRAM") as dscr:

            # fp32 transposed final hidden
            trh = pstr1.tile([128, 512], F32, tag="tr")
            for j in range(4):
                nc.tensor.transpose(trh[:, j * 128:(j + 1) * 128],
                                    h2f[:, j * 128:(j + 1) * 128], eye32_sb[:])
            hT32 = tailp.tile([128, 512], F32, tag="hT32")
            nc.vector.tensor_copy(hT32[:], trh[:])

            # queries q = h @ Wq.T + bq  [64, 512]
            q_ps = pst.tile([128, BN], F32, tag="pq")
            nc.tensor.matmul(q_ps[0:64, :], on32_sb[0:1, 0:64], bq_sb[0:1, :],
                             start=True, stop=False)
            for c in range(8):
                nc.tensor.matmul(q_ps[0:64, :], hT32[:, _hmap(c):_hmap(c) + 64],
                                 wq_sb[:, c, :], start=False, stop=(c == 7))
            q_sb = tailp.tile([128, BN], F32, tag="q")
            nc.vector.tensor_copy(q_sb[0:64, :], q_ps[0:64, :])

            # qT chunks: [128, 4, 64], chunk j partitions = (k=2j: c | k=2j+1: c)
            trq = pstr1.tile([128, 4, C], F32, tag="trq")
            for j in range(4):
                nc.tensor.transpose(trq[:, j, :], q_sb[0:64, j * 128:(j + 1) * 128],
                                    eye32_sb[0:64, 0:64])
            qT_sb = tailp.tile([128, 4, C], F32, tag="qT")
            nc.vector.tensor_copy(qT_sb[:], trq[:])

            # scores S_j = 2 q.e - |e|^2 - 1e-9 n  [128 (2 k), 1024 codes]
            idx_bounce = dscr.tile([1, 512], F32, tag="idxb")
            idxrow_t = tailp.tile([1, 512], F32, tag="idxrow")
            for j in range(4):
                S_ps = pst.tile([128, NCODE], F32, tag="pt")
                for ns in range(2):
                    nc.tensor.matmul(
                        S_ps[:, ns * 512:(ns + 1) * 512],
                        on32_sb[0:1, 0:128],
                        nn_sb[0:1, ns * 512:(ns + 1) * 512],
                        start=True, stop=False, skip_group_check=True)
                    for col in range(2):
                        po = col * 64
                        nc.tensor.matmul(
                            S_ps[po:po + 64, ns * 512:(ns + 1) * 512],
                            qT_sb[po:po + 64, j, :],
                            embT2_sb[po:po + 64, ns * 512:(ns + 1) * 512],
                            start=False, stop=True, skip_group_check=True,
                            tile_position=(po, po))
                S_sb = tailp.tile([128, NCODE], F32, tag="S")
                nc.vector.tensor_copy(S_sb[:], S_ps[:])
                mx = tailp.tile([128, 8], F32, tag="mx")
                mi = tailp.tile([128, 8], U32, tag="mi")
                nc.vector.max_with_indices(mx[:], mi[:], S_sb[:])
                nc.sync.dma_start(o_idx[:, 2 * j:2 * j + 1], mi[0:64, 0:1].bitcast(I32))
                nc.sync.dma_start(o_idx[:, 2 * j + 1:2 * j + 2], mi[64:128, 0:1].bitcast(I32))
                idxf = tailp.tile([128, 1], F32, tag="idxf")
                nc.vector.tensor_copy(idxf[:], mi[:, 0:1])
                nc.sync.dma_start(idx_bounce[0:1, j * 128:(j + 1) * 128], idxf[:, 0:1])

            idxrow = tailp.tile([1, 512], F32, tag="idxrow")
            nc.sync.dma_start(idxrow[:], idx_bounce[:])

            # broadcast idx over partitions; q index order along free = (k, b)
            bc_ps = pst.tile([128, 512], F32, tag="pq")
            nc.tensor.matmul(bc_ps[:], on32_sb[0:1, :], idxrow[0:1, :],
                             start=True, stop=True)
            bc_sb = dyng.tile([128, 512], F32, tag="gate")
            nc.vector.tensor_copy(bc_sb[:], bc_ps[:])

            # one-hot matmul -> narr [64, (k c)] (exact fp32 emb rows)
            narr_ps = pst.tile([128, BN], F32, tag="pq")
            nc.tensor.matmul(narr_ps[0:64, :], on32_sb[0:1, 0:64], zrow32[0:1, :],
                             start=True, stop=False)
            for nci in range(8):
                oh = dyng.tile([128, 512], F32, tag="gate")
                nc.vector.tensor_scalar(oh[:], bc_sb[:], iota_sb[:, nci:nci + 1], None,
                                        op0=OP.is_equal)
                for k in range(8):
                    nc.tensor.matmul(narr_ps[0:64, k * C:(k + 1) * C],
                                     oh[:, k * 64:(k + 1) * 64], emb8_sb[:, nci, :],
                                     start=False, stop=(nci == 7 and k == 7))
            narr_sb = tailp.tile([128, BN], F32, tag="narr")
            nc.vector.tensor_copy(narr_sb[0:64, :], narr_ps[0:64, :])
            nc.sync.dma_start(o_qst.rearrange("b k c -> b (k c)"), narr_sb[0:64, :])
            nc.sync.dma_start(o_narr[:], narr_sb[0:64, :])

            # vq loss partial: sum((q - narr)^2)
            d_sb = dyng.tile([128, BN], F32, tag="gate")
            nc.vector.tensor_tensor(d_sb[0:64, :], q_sb[0:64, :], narr_sb[0:64, :],
                                    OP.subtract)
            d2 = dyng.tile([128, BN], F32, tag="gate")
            dcol = tailp.tile([128, 1], F32, tag="dcol")
            nc.vector.scalar_tensor_tensor(d2[0:64, :], d_sb[0:64, :], 0.0,
                                           d_sb[0:64, :], op0=OP.add, op1=OP.mult,
                                           accum_out=dcol[0:64, :])
            loss_ps = pst.tile([128, 512], F32, tag="pq")
            nc.tensor.matmul(loss_ps[0:1, 0:1], dcol[0:64, 0:1], onc_sb[0:64, 0:1],
                             start=True, stop=True)
            loss_sb = tailp.tile([1, 1], F32, tag="loss")
            nc.vector.tensor_copy(loss_sb[:], loss_ps[0:1, 0:1])
            nc.sync.dma_start(o_loss[:], loss_sb[:])

            # narrT fp16 chunks for the heads
            narr16 = tailp.tile([128, BN], F16, tag="narr16")
            nc.scalar.copy(narr16[0:64, :], narr_sb[0:64, :])
            trn = pstr2.tile([128, 4, C], F16, tag="tr16")
            for j in range(4):
                nc.tensor.transpose(trn[:, j, :], narr16[0:64, j * 128:(j + 1) * 128],
                                    eye16_sb[0:64, 0:64])
            narrT = tailp.tile([128, 4, C], F16, tag="narrT")
            nc.vector.tensor_copy(narrT[:], trn[:])

            def head_layer(out_ps, in_chunks, w_sb_, b_sb_, nslices, nk):
                for ns in range(nslices):
                    nc.tensor.matmul(out_ps[0:64, ns * 512:(ns + 1) * 512],
                                     on16_sb[0:1, 0:64],
                                     b_sb_[0:1, ns * 512:(ns + 1) * 512],
                                     start=True, stop=False)
                    for c in range(nk):
                        nc.tensor.matmul(out_ps[0:64, ns * 512:(ns + 1) * 512],
                                         in_chunks(c),
                                         w_sb_[:, c, ns * 512:(ns + 1) * 512],
                                         start=False, stop=(c == nk - 1))

            def transpose8(src_sb, tag):
                """[64, 1024] f16 -> [128, 8, 64] f16 chunks."""
                tr_a = pstr2.tile([128, 4, C], F16, tag="tr16")
                for j in range(4):
                    nc.tensor.transpose(tr_a[:, j, :],
                                        src_sb[0:64, j * 128:(j + 1) * 128],
                                        eye16_sb[0:64, 0:64])
                tr_b = pstr2.tile([128, 4, C], F16, tag="tr16")
                for j in range(4):
                    nc.tensor.transpose(tr_b[:, j, :],
                                        src_sb[0:64, 512 + j * 128:512 + (j + 1) * 128],
                                        eye16_sb[0:64, 0:64])
                dst = tailp.tile([128, 8, C], F16, tag=tag)
                nc.vector.tensor_copy(dst[:, 0:4, :], tr_a[:])
                nc.vector.tensor_copy(dst[:, 4:8, :], tr_b[:])
                return dst

            # uncertainty head: silu(u_in @ uW1.T + ub1) @ uW2.T + ub2
            u1_ps = pst.tile([128, H], F32, tag="pt")
            head_layer(u1_ps,
                       lambda c: lw_sb[:, c, :] if c < 4 else narrT[:, c - 4, :],
                       uw1_sb, ub1_sb, 2, 8)
            s1 = tailp.tile([128, H], F16, tag="s1")
            nc.scalar.activation(s1[0:64, :], u1_ps[0:64, :], AF.Silu)
            s1T = transpose8(s1, "s1T")
            u2_ps = pst.tile([128, H], F32, tag="pt")
            head_layer(u2_ps, lambda c: s1T[:, c, :], uw2_sb, ub2_sb, 2, 8)
            u2_sb = tailp.tile([128, H], F32, tag="u2")
            nc.vector.tensor_copy(u2_sb[0:64, :], u2_ps[0:64, :])
            nc.sync.dma_start(o_unc[:], u2_sb[0:64, :])

            # prediction head: silu(narr @ pW1.T + pb1) @ pW2.T + pb2
            p1_ps = pst.tile([128, H], F32, tag="pt")
            head_layer(p1_ps, lambda c: narrT[:, c, :], pw1_sb, pb1_sb, 2, 4)
            s2 = tailp.tile([128, H], F16, tag="s2")
            nc.scalar.activation(s2[0:64, :], p1_ps[0:64, :], AF.Silu)
            s2T = transpose8(s2, "s2T")
            p2_ps = pst.tile([128, BN], F32, tag="pq")
            head_layer(p2_ps, lambda c: s2T[:, c, :], pw2_sb, pb2_sb, 1, 8)
            p2_sb = tailp.tile([128, BN], F32, tag="p2")
            nc.vector.tensor_copy(p2_sb[0:64, :], p2_ps[0:64, :])
            nc.sync.dma_start(o_pred[:], p2_sb[0:64, :])

    nc.compile()
    return nc


def _chunk3(M):
    """[n*128, X] -> [128, n, X] contiguous."""
    n = M.shape[0] // 128
    return np.ascontiguousarray(M.reshape(n, 128, -1).transpose(1, 0, 2))


def _prep_shared(inputs):
    f16 = np.float16
    f32 = np.float32
    W_ih = np.asarray(inputs["W_ih"], f32)
    W_hh = np.asarray(inputs["W_hh"], f32)
    b_ih = np.asarray(inputs["b_ih"], f32)
    b_hh = np.asarray(inputs["b_hh"], f32)
    Wq = np.asarray(inputs["Wq"], f32)
    bq = np.asarray(inputs["bq"], f32)
    emb = np.asarray(inputs["emb"], f32)
    uW1 = np.asarray(inputs["uW1"], f32)
    uW2 = np.asarray(inputs["uW2"], f32)
    pW1 = np.asarray(inputs["pW1"], f32)
    pW2 = np.asarray(inputs["pW2"], f32)
    p = np.arange(128, dtype=f32)

    def bt(vec):
        # [1024] bias -> [128, 512] broadcast tile (col-split layout)
        out = np.empty((128, 512), f32)
        out[0:64, :] = vec[None, 0:512]
        out[64:128, :] = vec[None, 512:1024]
        return out

    brz = b_ih + b_hh
    wihT = W_ih.T  # [512, 3072]
    whhT = W_hh.T  # [1024, 3072]
    wih_hi = wihT.astype(f16)
    wih_lo = (wihT - wih_hi.astype(f32)).astype(f16)
    whh_hi = whhT.astype(f16)
    whh_lo = (whhT - whh_hi.astype(f32)).astype(f16)
    return {
        "wihT_hi": _chunk3(wih_hi),
        "wihT_lo": _chunk3(wih_lo),
        "whhT_hi": _chunk3(whh_hi),
        "whhT_lo": _chunk3(whh_lo),
        "bt_r": bt(brz[0:1024]),
        "bt_z": bt(brz[1024:2048]),
        "bt_nh": bt(b_hh[2048:3072]),
        "bt_ni": bt(b_ih[2048:3072]),
        "wqT": _chunk3(Wq.T).astype(f32),
        "bq_row": bq[None, :].astype(f32),
        "embT2": np.concatenate([2.0 * emb.T, 2.0 * emb.T], axis=0).astype(f32),
        "negnorm": (-(emb.astype(np.float64) ** 2).sum(1)
                    - 1e-9 * np.arange(NCODE))[None, :].astype(f32),
        "emb8": _chunk3(emb).astype(f32),
        "uw1T": _chunk3(uW1.T).astype(f16),
        "ub1_row": np.asarray(inputs["ub1"], f32)[None, :].astype(f16),
        "uw2T": _chunk3(uW2.T).astype(f16),
        "ub2_row": np.asarray(inputs["ub2"], f32)[None, :].astype(f16),
        "pw1T": _chunk3(pW1.T).astype(f16),
        "pb1_row": np.asarray(inputs["pb1"], f32)[None, :].astype(f16),
        "pw2T": _chunk3(pW2.T).astype(f16),
        "pb2_row": np.asarray(inputs["pb2"], f32)[None, :].astype(f16),
        "ones16": np.ones((1, 128), f16),
        "ones32": np.ones((1, 128), f32),
        "onescol": np.ones((128, 1), f32),
        "eye16": np.eye(128, dtype=f16),
        "eye32": np.eye(128, dtype=f32),
        "iota8": (p[:, None] + 128.0 * np.arange(8, dtype=f32)[None, :]).astype(f32),
    }


def kernel(**inputs):
    if "nc" not in _CACHE:
        _CACHE["nc"] = _build()
    nc = _CACHE["nc"]

    x = np.asarray(inputs["state_window"], np.float32)
    shared = _prep_shared(inputs)
    in_maps = []
    for ci in range(NCORES):
        shard = x[ci * BS:(ci + 1) * BS]          # [64, 64, 512]
        xt = shard.transpose(2, 1, 0)             # [512 d, 64 w, 64 b]
        m = dict(shared)
        xflat = np.ascontiguousarray(xt.reshape(D, W * BS))
        x_hi = xflat.astype(np.float16)
        x_lo = (xflat - x_hi.astype(np.float32)).astype(np.float16)
        m["xT_hi"] = _chunk3(x_hi)
        m["xT_lo"] = _chunk3(x_lo)
        m["lwT16"] = _chunk3(np.ascontiguousarray(shard[:, -1, :].T)).astype(np.float16)
        in_maps.append(m)

    res = run_bass_kernel_spmd(nc, in_maps, list(range(NCORES)))
    kernel.LAST_RESULT = res

    r = res.results
    code_indices = np.concatenate([r[c]["o_idx"] for c in range(NCORES)], axis=0)
    quantized_st = np.concatenate([r[c]["o_qst"] for c in range(NCORES)], axis=0)
    narrator = np.concatenate([r[c]["o_narr"] for c in range(NCORES)], axis=0)
    uncertainty = np.concatenate([r[c]["o_unc"] for c in range(NCORES)], axis=0)
    predicted = np.concatenate([r[c]["o_pred"] for c in range(NCORES)], axis=0)
    last_hidden = np.concatenate([r[c]["o_hid"] for c in range(NCORES)], axis=0)
    total = sum(float(r[c]["o_loss"][0, 0]) for c in range(NCORES))
    vq_loss = np.float32(1.25 * total / (B * K * C))
    return (code_indices.astype(np.int32), quantized_st, narrator, uncertainty,
            predicted, vq_loss, last_hidden)


# revision 22
# speedup vs baseline: 1.3844x; 1.0066x over previous
"""Trainium2 Bass kernel for nn_DiscreteNarrator (GRU + VQ codebook + heads).

Strategy: data-parallel over batch across 8 NeuronCores (64 rows/core).
Per core:
  - GRU recurrence runs fully on-chip. Layout: gh[b, g] with batch on
    PSUM partitions, gate slices on the free dim; dual column-group
    matmuls (tile_position) pack two independent 512-wide gate slices
    into one 128x128 PE pass so the 64-row batch uses the full array.
  - Matmul inputs in fp16 (10-bit mantissa; verified 0 argmin flips vs
    fp32 reference on the generated codebook margins), fp32 PSUM
    accumulation. The x-projection for step t+1 is fused into step t's
    PE stream (no separate gi_all phase, no DRAM staging).
  - VQ tail in fp32: queries, scores S = 2 q.e - |e|^2 - 1e-9*n (argmax
    == argmin-with-first-index-tie-break), one-hot matmul against the
    fp32 codebook so quantized rows are exact; heads use fp16 weights.
vq_loss partial sums are reduced on the host (the only cross-core term).
"""
import os
import sys
import types

import numpy as np

_HERE = os.path.dirname(os.path.abspath(__file__))


def _register_ntff_hook():
    """Make trace=True (BASS_TRACE=1) work under axon if the hook is absent."""
    try:
        from antenv.axon_hooks import get_axon_ntff_profile_hook  # noqa
        return
    except Exception:
        pass
    try:
        from trn_agent_boot.trn_boot import _ntff_profile_via_ctypes
        hook = _ntff_profile_via_ctypes("/opt/axon/libaxon_pjrt.so")
        mod = types.ModuleType("antenv.axon_hooks")
        mod.get_axon_ntff_profile_hook = lambda: hook
        mod.set_axon_ntff_profile_hook = lambda h: None
        import antenv
        sys.modules["antenv.axon_hooks"] = mod
        antenv.axon_hooks = mod
    except Exception:
        pass


_register_ntff_hook()

import concourse.tile as tile
import concourse.mybir as mybir
from concourse import bacc
from concourse.bass_utils import run_bass_kernel_spmd

F32 = mybir.dt.float32
F16 = mybir.dt.float16
U32 = mybir.dt.uint32
I32 = mybir.dt.int32
AF = mybir.ActivationFunctionType
OP = mybir.AluOpType

# problem dims (hardcoded per contract)
B, W, D, H = 512, 64, 512, 1024
NCODE, K, C = 1024, 8, 64
BN = K * C          # 512
G3 = 3 * H          # 3072
NCORES = 8
BS = B // NCORES    # 64

_CACHE = {}


def _hmap(c):
    """Free-dim offset of hT chunk c inside the [128, 512] hT tile."""
    return (c % 4) * 128 + (c // 4) * 64


def _build():
    from contextlib import ExitStack

    nc = bacc.Bacc(None, target_bir_lowering=False)

    # ---- inputs ----
    xT_hi = nc.dram_tensor("xT_hi", [128, 4, W * BS], F16, kind="ExternalInput")
    xT_lo = nc.dram_tensor("xT_lo", [128, 4, W * BS], F16, kind="ExternalInput")
    wihT_hi = nc.dram_tensor("wihT_hi", [128, 4, G3], F16, kind="ExternalInput")
    wihT_lo = nc.dram_tensor("wihT_lo", [128, 4, G3], F16, kind="ExternalInput")
    whhT_hi = nc.dram_tensor("whhT_hi", [128, 8, G3], F16, kind="ExternalInput")
    whhT_lo = nc.dram_tensor("whhT_lo", [128, 8, G3], F16, kind="ExternalInput")
    bt_r = nc.dram_tensor("bt_r", [128, 512], F32, kind="ExternalInput")
    bt_z = nc.dram_tensor("bt_z", [128, 512], F32, kind="ExternalInput")
    bt_nh = nc.dram_tensor("bt_nh", [128, 512], F32, kind="ExternalInput")
    bt_ni = nc.dram_tensor("bt_ni", [128, 512], F32, kind="ExternalInput")
    wqT = nc.dram_tensor("wqT", [128, 8, BN], F32, kind="ExternalInput")
    bq_row = nc.dram_tensor("bq_row", [1, BN], F32, kind="ExternalInput")
    embT2 = nc.dram_tensor("embT2", [128, NCODE], F32, kind="ExternalInput")
    negnorm = nc.dram_tensor("negnorm", [1, NCODE], F32, kind="ExternalInput")
    emb8 = nc.dram_tensor("emb8", [128, 8, C], F32, kind="ExternalInput")
    lwT16 = nc.dram_tensor("lwT16", [128, 4, BS], F16, kind="ExternalInput")
    uw1T = nc.dram_tensor("uw1T", [128, 8, H], F16, kind="ExternalInput")
    ub1_row = nc.dram_tensor("ub1_row", [1, H], F16, kind="ExternalInput")
    uw2T = nc.dram_tensor("uw2T", [128, 8, H], F16, kind="ExternalInput")
    ub2_row = nc.dram_tensor("ub2_row", [1, H], F16, kind="ExternalInput")
    pw1T = nc.dram_tensor("pw1T", [128, 4, H], F16, kind="ExternalInput")
    pb1_row = nc.dram_tensor("pb1_row", [1, H], F16, kind="ExternalInput")
    pw2T = nc.dram_tensor("pw2T", [128, 8, BN], F16, kind="ExternalInput")
    pb2_row = nc.dram_tensor("pb2_row", [1, BN], F16, kind="ExternalInput")
    ones16 = nc.dram_tensor("ones16", [1, 128], F16, kind="ExternalInput")
    ones32 = nc.dram_tensor("ones32", [1, 128], F32, kind="ExternalInput")
    onescol = nc.dram_tensor("onescol", [128, 1], F32, kind="ExternalInput")
    eye16 = nc.dram_tensor("eye16", [128, 128], F16, kind="ExternalInput")
    eye32 = nc.dram_tensor("eye32", [128, 128], F32, kind="ExternalInput")
    iota8 = nc.dram_tensor("iota8", [128, 8], F32, kind="ExternalInput")

    # ---- outputs ----
    o_idx = nc.dram_tensor("o_idx", [BS, K], I32, kind="ExternalOutput")
    o_qst = nc.dram_tensor("o_qst", [BS, K, C], F32, kind="ExternalOutput")
    o_narr = nc.dram_tensor("o_narr", [BS, BN], F32, kind="ExternalOutput")
    o_unc = nc.dram_tensor("o_unc", [BS, H], F32, kind="ExternalOutput")
    o_pred = nc.dram_tensor("o_pred", [BS, BN], F32, kind="ExternalOutput")
    o_loss = nc.dram_tensor("o_loss", [1, 1], F32, kind="ExternalOutput")
    o_hid = nc.dram_tensor("o_hid", [BS, H], F32, kind="ExternalOutput")

    with tile.TileContext(nc) as tc, ExitStack() as stack:
        cpool = stack.enter_context(tc.tile_pool(name="cpool", bufs=1))
        dyn2 = stack.enter_context(tc.tile_pool(name="dyn2", bufs=2))
        dyng = stack.enter_context(tc.tile_pool(name="dyng", bufs=8))
        dynx = stack.enter_context(tc.tile_pool(name="dynx", bufs=3))

        # consts
        on16_sb = cpool.tile([1, 128], F16, tag="on16")
        on32_sb = cpool.tile([1, 128], F32, tag="on32")
        onc_sb = cpool.tile([128, 1], F32, tag="onc")
        eye16_sb = cpool.tile([128, 128], F16, tag="eye16")
        eye32_sb = cpool.tile([128, 128], F32, tag="eye32")
        iota_sb = cpool.tile([128, 8], F32, tag="iota")
        btr_sb = cpool.tile([128, 512], F32, tag="btr")
        btz_sb = cpool.tile([128, 512], F32, tag="btz")
        btnh_sb = cpool.tile([128, 512], F32, tag="btnh")
        btni_sb = cpool.tile([128, 512], F32, tag="btni")
        zrow16 = cpool.tile([1, 512], F16, tag="zrow16")
        nc.vector.memset(zrow16[:], 0.0)
        zrow32 = cpool.tile([1, 512], F32, tag="zrow32")
        nc.vector.memset(zrow32[:], 0.0)
        for dst, src in ((on16_sb, ones16), (on32_sb, ones32), (onc_sb, onescol),
                         (eye16_sb, eye16), (eye32_sb, eye32), (iota_sb, iota8),
                         (btr_sb, bt_r), (btz_sb, bt_z), (btnh_sb, bt_nh),
                         (btni_sb, bt_ni)):
            nc.sync.dma_start(dst[:], src[:])

        # ---------------- recurrence ----------------
        h2_prev = dyn2.tile([128, 512], F32, tag="h2")
        nc.vector.memset(h2_prev[:], 0.0)

        xt_tiles = {}

        def prefetch_xt(t):
            if t > W:
                return
            xh = dynx.tile([128, 4, BS], F16, tag="xth")
            nc.gpsimd.dma_start(xh[:], xT_hi[:, :, (t - 1) * BS:t * BS])
            xl = dynx.tile([128, 4, BS], F16, tag="xtl")
            nc.gpsimd.dma_start(xl[:], xT_lo[:, :, (t - 1) * BS:t * BS])
            xt_tiles[t] = (xh, xl)

        with tc.tile_pool(name="rw", bufs=1) as rw:
            wih_hi = rw.tile([128, 4, G3], F16, tag="wihhi")
            wih_lo = rw.tile([128, 4, G3], F16, tag="wihlo")
            whh_hi = rw.tile([128, 8, G3], F16, tag="whhhi")
            whh_lo = rw.tile([128, 8, G3], F16, tag="whhlo")
            nc.sync.dma_start(wih_hi[:], wihT_hi[:])
            nc.sync.dma_start(wih_lo[:], wihT_lo[:])
            nc.sync.dma_start(whh_hi[:], whhT_hi[:])
            nc.sync.dma_start(whh_lo[:], whhT_lo[:])

            with tc.tile_pool(name="ghp", bufs=2, space="PSUM") as ghp, \
                 tc.tile_pool(name="smp", bufs=2, space="PSUM") as smp:

                def a_block(t, first_step):
                    """x-projection matmuls (3-term hi/lo) for step t."""
                    xh, xl = xt_tiles.pop(t)
                    gh = ghp.tile([128, 1536], F32, tag="gh")
                    i_n = smp.tile([128, 512], F32, tag="sm")
                    o128 = on16_sb[0:1, 0:128]
                    # one start=True zeroing matmul per bank region
                    nc.tensor.matmul(gh[:, 0:512], o128, zrow16[0:1, :],
                                     start=True, stop=False, skip_group_check=True)
                    nc.tensor.matmul(gh[:, 512:1024], o128, zrow16[0:1, :],
                                     start=True, stop=False, skip_group_check=True)
                    nc.tensor.matmul(gh[:, 1024:1536], o128, zrow16[0:1, :],
                                     start=True, stop=first_step, skip_group_check=True)
                    nc.tensor.matmul(i_n[:, :], o128, zrow16[0:1, :],
                                     start=True, stop=False, skip_group_check=True)
                    terms = ((xh, wih_hi), (xh, wih_lo), (xl, wih_hi))
                    for ti, (xa, wa) in enumerate(terms):
                        for c in range(4):
                            last = (ti == 2 and c == 3)
                            for g0, dst, lo_f, stop in (
                                    (0, gh, 0, last and first_step),
                                    (1024, gh, 512, last and first_step),
                                    (2048, i_n, 0, last)):
                                for col in range(2):
                                    po = col * 64
                                    cs = col * 512
                                    nc.tensor.matmul(
                                        dst[po:po + 64, lo_f:lo_f + 512],
                                        xa[:, c, :],
                                        wa[:, c, g0 + cs:g0 + cs + 512],
                                        start=False, stop=stop,
                                        skip_group_check=True,
                                        tile_position=(0, po))
                    return gh, i_n

                def b_block(gh, hT_prev):
                    """h-recurrence matmuls (3-term hi/lo). Region order: r, hn, z."""
                    def hchunk(c, lo):
                        off = 512 * lo + _hmap(c)
                        return hT_prev[:, off:off + 64]
                    for g0, fr in ((0, 0), (2048, 1024), (1024, 512)):
                        for ti in range(3):
                            wa = whh_lo if ti == 1 else whh_hi
                            hlo = 1 if ti == 2 else 0
                            for c in range(8):
                                hc = hchunk(c, hlo)
                                for col in range(2):
                                    po = col * 64
                                    nc.tensor.matmul(
                                        gh[po:po + 64, fr:fr + 512], hc,
                                        wa[:, c, g0 + col * 512:g0 + col * 512 + 512],
                                        start=False,
                                        stop=(ti == 2 and c == 7),
                                        skip_group_check=True,
                                        tile_position=(0, po))

                def c_block(gh, i_n, h2_in, t):
                    last = (t == W)
                    r_pre = dyng.tile([128, 512], F32, tag="gate")
                    nc.vector.tensor_tensor(r_pre[:], gh[:, 0:512], btr_sb[:], OP.add)
                    r_sb = dyng.tile([128, 512], F32, tag="gate")
                    nc.scalar.activation(r_sb[:], r_pre[:], AF.Sigmoid)
                    t2a = dyng.tile([128, 512], F32, tag="gate")
                    nc.vector.tensor_tensor(t2a[:], i_n[:], btni_sb[:], OP.add)
                    t1 = dyng.tile([128, 512], F32, tag="gate")
                    if t == 1:
                        nc.vector.tensor_tensor(t1[:], btnh_sb[:], r_sb[:], OP.mult)
                    else:
                        t0 = dyng.tile([128, 512], F32, tag="gate")
                        nc.vector.tensor_tensor(t0[:], gh[:, 1024:1536], btnh_sb[:], OP.add)
                        nc.vector.tensor_tensor(t1[:], t0[:], r_sb[:], OP.mult)
                    t2 = dyng.tile([128, 512], F32, tag="gate")
                    nc.vector.tensor_tensor(t2[:], t1[:], t2a[:], OP.add)
                    n_sb = dyng.tile([128, 512], F32, tag="gate")
                    nc.scalar.activation(n_sb[:], t2[:], AF.Tanh)
                    z_pre = dyng.tile([128, 512], F32, tag="gate")
                    nc.vector.tensor_tensor(z_pre[:], gh[:, 512:1024], btz_sb[:], OP.add)
                    z_sb = dyng.tile([128, 512], F32, tag="gate")
                    nc.scalar.activation(z_sb[:], z_pre[:], AF.Sigmoid)
                    v_sb = dyng.tile([128, 512], F32, tag="gate")
                    nc.scalar.activation(v_sb[:], z_pre[:], AF.Sigmoid, scale=-1.0)
                    u_sb = dyng.tile([128, 512], F32, tag="gate")
                    nc.vector.tensor_tensor(u_sb[:], z_sb[:], h2_in[:], OP.mult)
                    w_sb = dyng.tile([128, 512], F32, tag="gate")
                    nc.vector.tensor_tensor(w_sb[:], v_sb[:], n_sb[:], OP.mult)
                    h2_t = dyn2.tile([128, 512], F32, tag="h2")
                    nc.vector.tensor_tensor(h2_t[:], u_sb[:], w_sb[:], OP.add)
                    if last:
                        return h2_t, None, None
                    h2b_hi = dyn2.tile([128, 512], F16, tag="h2bh")
                    nc.scalar.copy(h2b_hi[:], h2_t[:])
                    h2b_lo = dyn2.tile([128, 512], F16, tag="h2bl")
                    nc.vector.tensor_tensor(h2b_lo[:], h2_t[:], h2b_hi[:], OP.subtract)
                    return h2_t, h2b_hi, h2b_lo

                def d_block(h2b_hi, h2b_lo):
                    tr = smp.tile([128, 1024], F16, tag="sm")
                    for j in range(4):
                        nc.tensor.transpose(tr[:, j * 128:(j + 1) * 128],
                                            h2b_hi[:, j * 128:(j + 1) * 128], eye16_sb[:])
                    for j in range(4):
                        nc.tensor.transpose(tr[:, 512 + j * 128:512 + (j + 1) * 128],
                                            h2b_lo[:, j * 128:(j + 1) * 128], eye16_sb[:])
                    hT_t = dyn2.tile([128, 1024], F16, tag="hT")
                    nc.vector.tensor_copy(hT_t[:], tr[:])
                    return hT_t

                prefetch_xt(1)
                prefetch_xt(2)
                gh_t, in_t = a_block(1, True)
                hT_prev = None
                for t in range(1, W + 1):
                    prefetch_xt(t + 2)
                    if t > 1:
                        b_block(gh_t, hT_prev)
                    h2_t, h2b_hi, h2b_lo = c_block(gh_t, in_t, h2_prev, t)
                    if t < W:
                        gh_t, in_t = a_block(t + 1, False)
                        hT_prev = d_block(h2b_hi, h2b_lo)
                    h2_prev = h2_t

        h2f = h2_prev  # [128, 512] f32: p<64 -> h[b, 0:512]; p>=64 -> h[b, 512:1024]
        nc.sync.dma_start(o_hid[:, 0:512], h2f[0:64, :])
        nc.sync.dma_start(o_hid[:, 512:1024], h2f[64:128, :])

        # tail weights (allocated after the recurrence weights free their space)
        tailp = stack.enter_context(tc.tile_pool(name="tailp", bufs=1))
        wq_sb = tailp.tile([128, 8, BN], F32, tag="wq")
        uw1_sb = tailp.tile([128, 8, H], F16, tag="uw1")
        embT2_sb = tailp.tile([128, NCODE], F32, tag="embT2")
        nn_sb = tailp.tile([1, NCODE], F32, tag="negnorm")
        emb8_sb = tailp.tile([128, 8, C], F32, tag="emb8")
        lw_sb = tailp.tile([128, 4, BS], F16, tag="lw")
        uw2_sb = tailp.tile([128, 8, H], F16, tag="uw2")
        pw1_sb = tailp.tile([128, 4, H], F16, tag="pw1")
        pw2_sb = tailp.tile([128, 8, BN], F16, tag="pw2")
        bq_sb = tailp.tile([1, BN], F32, tag="bq")
        ub1_sb = tailp.tile([1, H], F16, tag="ub1")
        ub2_sb = tailp.tile([1, H], F16, tag="ub2")
        pb1_sb = tailp.tile([1, H], F16, tag="pb1")
        pb2_sb = tailp.tile([1, BN], F16, tag="pb2")
        for dst, src in ((wq_sb, wqT), (uw1_sb, uw1T), (embT2_sb, embT2),
                         (nn_sb, negnorm), (emb8_sb, emb8), (lw_sb, lwT16),
                         (uw2_sb, uw2T), (pw1_sb, pw1T), (pw2_sb, pw2T),
                         (bq_sb, bq_row), (ub1_sb, ub1_row), (ub2_sb, ub2_row),
                         (pb1_sb, pb1_row), (pb2_sb, pb2_row)):
            nc.sync.dma_start(dst[:], src[:])

        # ---------------- tail ----------------
        with tc.tile_pool(name="pstail", bufs=1, space="PSUM") as pst, \
             tc.tile_pool(name="psbig", bufs=2, space="PSUM") as psbig, \
             tc.tile_pool(name="pstr1", bufs=1, space="PSUM") as pstr1, \
             tc.tile_pool(name="pstr2", bufs=2, space="PSUM") as pstr2, \
             tc.tile_pool(name="dscr", bufs=1, space="DRAM") as dscr:

            # fp32 transposed final hidden
            trh = pstr1.tile([128, 512], F32, tag="tr")
            for j in range(4):
                nc.tensor.transpose(trh[:, j * 128:(j + 1) * 128],
                                    h2f[:, j * 128:(j + 1) * 128], eye32_sb[:])
            hT32 = tailp.tile([128, 512], F32, tag="hT32")
            nc.vector.tensor_copy(hT32[:], trh[:])

            # queries q = h @ Wq.T + bq  [64, 512]
            q_ps = pst.tile([128, BN], F32, tag="pq")
            nc.tensor.matmul(q_ps[0:64, :], on32_sb[0:1, 0:64], bq_sb[0:1, :],
                             start=True, stop=False)
            for c in range(8):
                nc.tensor.matmul(q_ps[0:64, :], hT32[:, _hmap(c):_hmap(c) + 64],
                                 wq_sb[:, c, :], start=False, stop=(c == 7))
            q_sb = tailp.tile([128, BN], F32, tag="q")
            nc.vector.tensor_copy(q_sb[0:64, :], q_ps[0:64, :])

            # qT chunks: [128, 4, 64], chunk j partitions = (k=2j: c | k=2j+1: c)
            trq = pst.tile([128, 4, C], F32, tag="pq")
            for j in range(4):
                nc.tensor.transpose(trq[:, j, :], q_sb[0:64, j * 128:(j + 1) * 128],
                                    eye32_sb[0:64, 0:64])
            qT_sb = tailp.tile([128, 4, C], F32, tag="qT")
            nc.vector.tensor_copy(qT_sb[:], trq[:])

            # scores S_j = 2 q.e - |e|^2 - 1e-9 n  [128 (2 k), 1024 codes]
            idx_bounce = dscr.tile([1, 512], F32, tag="idxb")
            idxrow_t = tailp.tile([1, 512], F32, tag="idxrow")
            for j in range(4):
                S_ps = psbig.tile([128, NCODE], F32, tag="pt")
                for ns in range(2):
                    nc.tensor.matmul(
                        S_ps[:, ns * 512:(ns + 1) * 512],
                        on32_sb[0:1, 0:128],
                        nn_sb[0:1, ns * 512:(ns + 1) * 512],
                        start=True, stop=False, skip_group_check=True)
                    for col in range(2):
                        po = col * 64
                        nc.tensor.matmul(
                            S_ps[po:po + 64, ns * 512:(ns + 1) * 512],
                            qT_sb[po:po + 64, j, :],
                            embT2_sb[po:po + 64, ns * 512:(ns + 1) * 512],
                            start=False, stop=True, skip_group_check=True,
                            tile_position=(po, po))
                S_sb = tailp.tile([128, NCODE], F32, tag="S")
                nc.vector.tensor_copy(S_sb[:], S_ps[:])
                mx = tailp.tile([128, 8], F32, tag="mx")
                mi = tailp.tile([128, 8], U32, tag="mi")
                nc.vector.max_with_indices(mx[:], mi[:], S_sb[:])
                nc.sync.dma_start(o_idx[:, 2 * j:2 * j + 1], mi[0:64, 0:1].bitcast(I32))
                nc.sync.dma_start(o_idx[:, 2 * j + 1:2 * j + 2], mi[64:128, 0:1].bitcast(I32))
                idxf = tailp.tile([128, 1], F32, tag="idxf")
                nc.vector.tensor_copy(idxf[:], mi[:, 0:1])
                nc.sync.dma_start(idx_bounce[0:1, j * 128:(j + 1) * 128], idxf[:, 0:1])

            idxrow = tailp.tile([1, 512], F32, tag="idxrow")
            nc.sync.dma_start(idxrow[:], idx_bounce[:])

            # broadcast idx over partitions; q index order along free = (k, b)
            bc_ps = pst.tile([128, 512], F32, tag="pq")
            nc.tensor.matmul(bc_ps[:], on32_sb[0:1, :], idxrow[0:1, :],
                             start=True, stop=True)
            bc_sb = dyng.tile([128, 512], F32, tag="gate")
            nc.vector.tensor_copy(bc_sb[:], bc_ps[:])

            # one-hot matmul -> narr [64, (k c)] (exact fp32 emb rows)
            narr_ps = pst.tile([128, BN], F32, tag="pq")
            nc.tensor.matmul(narr_ps[0:64, :], on32_sb[0:1, 0:64], zrow32[0:1, :],
                             start=True, stop=False)
            for nci in range(8):
                oh = dyng.tile([128, 512], F32, tag="gate")
                nc.vector.tensor_scalar(oh[:], bc_sb[:], iota_sb[:, nci:nci + 1], None,
                                        op0=OP.is_equal)
                for k in range(8):
                    nc.tensor.matmul(narr_ps[0:64, k * C:(k + 1) * C],
                                     oh[:, k * 64:(k + 1) * 64], emb8_sb[:, nci, :],
                                     start=False, stop=(nci == 7 and k == 7))
            narr_sb = tailp.tile([128, BN], F32, tag="narr")
            nc.vector.tensor_copy(narr_sb[0:64, :], narr_ps[0:64, :])
            nc.sync.dma_start(o_qst.rearrange("b k c -> b (k c)"), narr_sb[0:64, :])
            nc.sync.dma_start(o_narr[:], narr_sb[0:64, :])

            # vq loss partial: sum((q - narr)^2)
            d_sb = dyng.tile([128, BN], F32, tag="gate")
            nc.vector.tensor_tensor(d_sb[0:64, :], q_sb[0:64, :], narr_sb[0:64, :],
                                    OP.subtract)
            d2 = dyng.tile([128, BN], F32, tag="gate")
            dcol = tailp.tile([128, 1], F32, tag="dcol")
            nc.vector.scalar_tensor_tensor(d2[0:64, :], d_sb[0:64, :], 0.0,
                                           d_sb[0:64, :], op0=OP.add, op1=OP.mult,
                                           accum_out=dcol[0:64, :])
            loss_ps = pst.tile([128, 512], F32, tag="pq")
            nc.tensor.matmul(loss_ps[0:1, 0:1], dcol[0:64, 0:1], onc_sb[0:64, 0:1],
                             start=True, stop=True)
            loss_sb = tailp.tile([1, 1], F32, tag="loss")
            nc.vector.tensor_copy(loss_sb[:], loss_ps[0:1, 0:1])
            nc.sync.dma_start(o_loss[:], loss_sb[:])

            # narrT fp16 chunks for the heads
            narr16 = tailp.tile([128, BN], F16, tag="narr16")
            nc.scalar.copy(narr16[0:64, :], narr_sb[0:64, :])
            trn = pstr2.tile([128, 4, C], F16, tag="tr16")
            for j in range(4):
                nc.tensor.transpose(trn[:, j, :], narr16[0:64, j * 128:(j + 1) * 128],
                                    eye16_sb[0:64, 0:64])
            narrT = tailp.tile([128, 4, C], F16, tag="narrT")
            nc.vector.tensor_copy(narrT[:], trn[:])

            def head_layer(out_ps, in_chunks, w_sb_, b_sb_, nslices, nk):
                for ns in range(nslices):
                    nc.tensor.matmul(out_ps[0:64, ns * 512:(ns + 1) * 512],
                                     on16_sb[0:1, 0:64],
                                     b_sb_[0:1, ns * 512:(ns + 1) * 512],
                                     start=True, stop=False)
                    for c in range(nk):
                        nc.tensor.matmul(out_ps[0:64, ns * 512:(ns + 1) * 512],
                                         in_chunks(c),
                                         w_sb_[:, c, ns * 512:(ns + 1) * 512],
                                         start=False, stop=(c == nk - 1))

            def transpose8(src_sb, tag):
                """[64, 1024] f16 -> [128, 8, 64] f16 chunks."""
                tr_a = pstr2.tile([128, 4, C], F16, tag="tr16")
                for j in range(4):
                    nc.tensor.transpose(tr_a[:, j, :],
                                        src_sb[0:64, j * 128:(j + 1) * 128],
                                        eye16_sb[0:64, 0:64])
                tr_b = pstr2.tile([128, 4, C], F16, tag="tr16")
                for j in range(4):
                    nc.tensor.transpose(tr_b[:, j, :],
                                        src_sb[0:64, 512 + j * 128:512 + (j + 1) * 128],
                                        eye16_sb[0:64, 0:64])
                dst = tailp.tile([128, 8, C], F16, tag=tag)
                nc.vector.tensor_copy(dst[:, 0:4, :], tr_a[:])
                nc.vector.tensor_copy(dst[:, 4:8, :], tr_b[:])
                return dst

            # uncertainty head: silu(u_in @ uW1.T + ub1) @ uW2.T + ub2
            u1_ps = psbig.tile([128, H], F32, tag="pt")
            head_layer(u1_ps,
                       lambda c: lw_sb[:, c, :] if c < 4 else narrT[:, c - 4, :],
                       uw1_sb, ub1_sb, 2, 8)
            s1 = tailp.tile([128, H], F16, tag="s1")
            nc.scalar.activation(s1[0:64, :], u1_ps[0:64, :], AF.Silu)
            s1T = transpose8(s1, "s1T")
            u2_ps = psbig.tile([128, H], F32, tag="pt")
            head_layer(u2_ps, lambda c: s1T[:, c, :], uw2_sb, ub2_sb, 2, 8)
            u2_sb = tailp.tile([128, H], F32, tag="u2")
            nc.vector.tensor_copy(u2_sb[0:64, :], u2_ps[0:64, :])
            nc.sync.dma_start(o_unc[:], u2_sb[0:64, :])

            # prediction head: silu(narr @ pW1.T + pb1) @ pW2.T + pb2
            p1_ps = psbig.tile([128, H], F32, tag="pt")
            head_layer(p1_ps, lambda c: narrT[:, c, :], pw1_sb, pb1_sb, 2, 4)
            s2 = tailp.tile([128, H], F16, tag="s2")
            nc.scalar.activation(s2[0:64, :], p1_ps[0:64, :], AF.Silu)
            s2T = transpose8(s2, "s2T")
            p2_ps = pst.tile([128, BN], F32, tag="pq")
            head_layer(p2_ps, lambda c: s2T[:, c, :], pw2_sb, pb2_sb, 1, 8)
            p2_sb = tailp.tile([128, BN], F32, tag="p2")
            nc.vector.tensor_copy(p2_sb[0:64, :], p2_ps[0:64, :])
            nc.sync.dma_start(o_pred[:], p2_sb[0:64, :])

    nc.compile()
    return nc


def _chunk3(M):
    """[n*128, X] -> [128, n, X] contiguous."""
    n = M.shape[0] // 128
    return np.ascontiguousarray(M.reshape(n, 128, -1).transpose(1, 0, 2))


def _prep_shared(inputs):
    f16 = np.float16
    f32 = np.float32
    W_ih = np.asarray(inputs["W_ih"], f32)
    W_hh = np.asarray(inputs["W_hh"], f32)
    b_ih = np.asarray(inputs["b_ih"], f32)
    b_hh = np.asarray(inputs["b_hh"], f32)
    Wq = np.asarray(inputs["Wq"], f32)
    bq = np.asarray(inputs["bq"], f32)
    emb = np.asarray(inputs["emb"], f32)
    uW1 = np.asarray(inputs["uW1"], f32)
    uW2 = np.asarray(inputs["uW2"], f32)
    pW1 = np.asarray(inputs["pW1"], f32)
    pW2 = np.asarray(inputs["pW2"], f32)
    p = np.arange(128, dtype=f32)

    def bt(vec):
        # [1024] bias -> [128, 512] broadcast tile (col-split layout)
        out = np.empty((128, 512), f32)
        out[0:64, :] = vec[None, 0:512]
        out[64:128, :] = vec[None, 512:1024]
        return out

    brz = b_ih + b_hh
    wihT = W_ih.T  # [512, 3072]
    whhT = W_hh.T  # [1024, 3072]
    wih_hi = wihT.astype(f16)
    wih_lo = (wihT - wih_hi.astype(f32)).astype(f16)
    whh_hi = whhT.astype(f16)
    whh_lo = (whhT - whh_hi.astype(f32)).astype(f16)
    return {
        "wihT_hi": _chunk3(wih_hi),
        "wihT_lo": _chunk3(wih_lo),
        "whhT_hi": _chunk3(whh_hi),
        "whhT_lo": _chunk3(whh_lo),
        "bt_r": bt(brz[0:1024]),
        "bt_z": bt(brz[1024:2048]),
        "bt_nh": bt(b_hh[2048:3072]),
        "bt_ni": bt(b_ih[2048:3072]),
        "wqT": _chunk3(Wq.T).astype(f32),
        "bq_row": bq[None, :].astype(f32),
        "embT2": np.concatenate([2.0 * emb.T, 2.0 * emb.T], axis=0).astype(f32),
        "negnorm": (-(emb.astype(np.float64) ** 2).sum(1)
                    - 1e-9 * np.arange(NCODE))[None, :].astype(f32),
        "emb8": _chunk3(emb).astype(f32),
        "uw1T": _chunk3(uW1.T).astype(f16),
        "ub1_row": np.asarray(inputs["ub1"], f32)[None, :].astype(f16),
        "uw2T": _chunk3(uW2.T).astype(f16),
        "ub2_row": np.asarray(inputs["ub2"], f32)[None, :].astype(f16),
        "pw1T": _chunk3(pW1.T).astype(f16),
        "pb1_row": np.asarray(inputs["pb1"], f32)[None, :].astype(f16),
        "pw2T": _chunk3(pW2.T).astype(f16),
        "pb2_row": np.asarray(inputs["pb2"], f32)[None, :].astype(f16),
        "ones16": np.ones((1, 128), f16),
        "ones32": np.ones((1, 128), f32),
        "onescol": np.ones((128, 1), f32),
        "eye16": np.eye(128, dtype=f16),
        "eye32": np.eye(128, dtype=f32),
        "iota8": (p[:, None] + 128.0 * np.arange(8, dtype=f32)[None, :]).astype(f32),
    }


def kernel(**inputs):
    if "nc" not in _CACHE:
        _CACHE["nc"] = _build()
    nc = _CACHE["nc"]

    x = np.asarray(inputs["state_window"], np.float32)
    shared = _prep_shared(inputs)
    in_maps = []
    for ci in range(NCORES):
        shard = x[ci * BS:(ci + 1) * BS]          # [64, 64, 512]
        xt = shard.transpose(2, 1, 0)             # [512 d, 64 w, 64 b]
        m = dict(shared)
        xflat = np.ascontiguousarray(xt.reshape(D, W * BS))
        x_hi = xflat.astype(np.float16)
        x_lo = (xflat - x_hi.astype(np.float32)).astype(np.float16)
        m["xT_hi"] = _chunk3(x_hi)
        m["xT_lo"] = _chunk3(x_lo)
        m["lwT16"] = _chunk3(np.ascontiguousarray(shard[:, -1, :].T)).astype(np.float16)
        in_maps.append(m)

    res = run_bass_kernel_spmd(nc, in_maps, list(range(NCORES)))
    kernel.LAST_RESULT = res

    r = res.results
    code_indices = np.concatenate([r[c]["o_idx"] for c in range(NCORES)], axis=0)
    quantized_st = np.concatenate([r[c]["o_qst"] for c in range(NCORES)], axis=0)
    narrator = np.concatenate([r[c]["o_narr"] for c in range(NCORES)], axis=0)
    uncertainty = np.concatenate([r[c]["o_unc"] for c in range(NCORES)], axis=0)
    predicted = np.concatenate([r[c]["o_pred"] for c in range(NCORES)], axis=0)
    last_hidden = np.concatenate([r[c]["o_hid"] for c in range(NCORES)], axis=0)
    total = sum(float(r[c]["o_loss"][0, 0]) for c in range(NCORES))
    vq_loss = np.float32(1.25 * total / (B * K * C))
    return (code_indices.astype(np.int32), quantized_st, narrator, uncertainty,
            predicted, vq_loss, last_hidden)
